# revision 20
# baseline (speedup 1.0000x reference)
"""Trainium2 Bass kernel for nn_AdaptiveResBlock (8-core data-parallel).

Reference computation (per batch element b, C=256 channels, T=8192 time):
  for i, dil in enumerate((1, 2, 4)):
      xt = lrelu(x)
      xP, xF = time-gather of xt at round(t -/+ d*dil), zero out-of-range
      xt = WC@xt + WP@xP + WF@xF + biases        (1x1 convs over channels)
      xt = lrelu(xt)
      xt = conv3(xt, WA) + bias
      x = xt + x

Dataflow (v3 — gather on GPSIMD, PE runs only dense matmuls):
  * gather commutes with the 1x1 convs: WP@gather(xt) == gather(WP@xt).
  * B-stage (PE, weights stationary): u = [WP|WF]@xt produced channel-major
    in PSUM; ACT packs it bf16 into SBUF as uP/uF tensors [128, T+1, 2]
    where the d=2 pair holds channel chunks (c, c+128) — both gathered with
    the same per-token index.
  * The time-gather runs on GPSIMD ap_gather (indices precomputed on host,
    +1-shifted; index 0 points at a zeroed pad column => free masking).
    P/F results are combined on GPSIMD; DVE adds them into the WC PSUM.
  * WC (PE, weights stationary) writes v channel-major; ACT applies
    leaky-relu straight from PSUM into rolling v tiles; conv3 is 6 dense
    matmuls per (ob, tile); DVE does the residual add and the next
    iteration's lrelu.
  * No PE transposes, no one-hot matmuls, no S matrices.

Sharded data-parallel over B=8 across the 8 NeuronCores; weights replicated.
"""

import numpy as np
import ml_dtypes
from contextlib import ExitStack

import concourse.bass as bass
import concourse.tile as tile
from concourse import mybir, bacc
from concourse.bass_utils import run_bass_kernel_spmd

F32 = mybir.dt.float32
BF16 = mybir.dt.bfloat16
I16 = mybir.dt.int16
AF = mybir.ActivationFunctionType
OP = mybir.AluOpType

B, C, T_FULL = 8, 256, 8192
DILATIONS = (1, 2, 4)
NITER = len(DILATIONS)
SLOPE = 0.1
LAG_G = 2   # gather runs LAG_G tiles behind B (u halo is a full step old)
LAG_V = 3   # v assembly (WC + pf inject + prelu) — pf is a full step old
LAG_C = 5   # conv3 — v tile and its halo are a full step old


def build_nc(T=T_FULL, num_devices=8):
    nT = T // 512
    NE = T + 1

    nc = bacc.Bacc("TRN2", target_bir_lowering=False, debug=False,
                   num_devices=num_devices)
    x_d = nc.declare_dram_parameter("x", [2, 128, T], F32, isOutput=False)
    wpf_d = nc.declare_dram_parameter("wpf", [NITER, 128, 2, 4, 128], BF16,
                                      isOutput=False)
    wct_d = nc.declare_dram_parameter("wct", [NITER, 128, 2, 2, 128], BF16,
                                      isOutput=False)
    wa_d = nc.declare_dram_parameter("wa", [NITER, 128, 3, 2, 2, 128], BF16,
                                     isOutput=False)
    b3_d = nc.declare_dram_parameter("b3", [NITER, 128, 2], F32,
                                     isOutput=False)
    id_d = nc.declare_dram_parameter("ident", [128, 128], BF16, isOutput=False)
    ix_d = nc.declare_dram_parameter("ix", [NITER, nT, 2, 128, 32], I16,
                                     isOutput=False)
    out_d = nc.declare_dram_parameter("out", [2, 128, T], F32, isOutput=True)

    with tile.TileContext(nc) as tc, ExitStack() as ctx:
        big = ctx.enter_context(tc.tile_pool(name="big", bufs=1))
        xtp = ctx.enter_context(tc.tile_pool(name="xtp", bufs=6))
        vtp = ctx.enter_context(tc.tile_pool(name="vtp", bufs=5))
        gpp = ctx.enter_context(tc.tile_pool(name="gpp", bufs=4))
        ixp = ctx.enter_context(tc.tile_pool(name="ixp", bufs=2))
        wts = ctx.enter_context(tc.tile_pool(name="wts", bufs=2))
        psp = ctx.enter_context(tc.tile_pool(name="psp", bufs=1, space="PSUM"))

        # ---- resident tensors ----
        x_sb = big.tile([128, 2, T], F32)        # fp32 residual signal
        uP = big.tile([128, NE, 2], BF16)        # u = WP@lrelu(x), packed
        uF = big.tile([128, NE, 2], BF16)        # (pair = channel chunks c, c+128)

        ix_sb = [None] * NITER

        def load_ix(i):
            t = ixp.tile([128, nT, 2, 32], I16, tag="ix")
            nc.sync.dma_start(t[:, :, :, :],
                              ix_d[i].rearrange("t h p s -> p t h s"))
            ix_sb[i] = t

        load_ix(0)
        ident_sb = big.tile([128, 128], BF16)
        nc.sync.dma_start(ident_sb[:, :], id_d[:, :])
        # x loads: per-tile, in consumption order
        for t8 in range(nT):
            sl = bass.ts(t8, 512)
            for cb in range(2):
                nc.sync.dma_start(x_sb[:, cb, sl], x_d[cb, :, sl])
        nc.vector.memset(uP[:, 0:1, :], 0.0)
        nc.vector.memset(uF[:, 0:1, :], 0.0)

        def load_weights(i):
            # all on the sync ring: a weight DMA on the ACT queue would
            # head-of-line block the activations behind its sem wait
            wpf_sb = wts.tile([128, 2, 4, 128], BF16, tag="wpf", bufs=3)
            nc.sync.dma_start(wpf_sb[:, :, :, :], wpf_d[i])
            wct_sb = wts.tile([128, 2, 2, 128], BF16, tag="wct", bufs=3)
            nc.sync.dma_start(wct_sb[:, :, :, :], wct_d[i])
            wa_sb = wts.tile([128, 3, 2, 2, 128], BF16, tag="wa", bufs=3)
            nc.sync.dma_start(wa_sb[:, :, :, :, :], wa_d[i])
            b3_sb = wts.tile([128, 2], F32, tag="b3", bufs=3)
            nc.sync.dma_start(b3_sb[:, :], b3_d[i])
            return wpf_sb, wct_sb, wa_sb, b3_sb

        w_cur = load_weights(0)
        w_next = None

        xt_tiles = [None] * nT
        xt_next = [None] * nT
        v_tiles = [None] * nT
        pf_tiles = [None] * nT

        def emit_A(tt):
            # xt = lrelu(x) on DVE, fp32 -> bf16
            tsl = bass.ts(tt, 512)
            xt = xtp.tile([128, 2, 512], BF16, tag="xt")
            nc.vector.scalar_tensor_tensor(
                xt[:, :, :], x_sb[:, :, tsl], SLOPE, x_sb[:, :, tsl],
                OP.mult, OP.max)
            return xt

        def emit_B(tt, wpf_sb):
            # u strips channel-major into PSUM; ACT packs into uP/uF
            for s in range(4):
                ups = psp.tile([128, 512], F32, tag=f"pu{s}")
                for cb in range(2):
                    nc.tensor.matmul(ups[:, :], wpf_sb[:, cb, s, :],
                                     xt_tiles[tt][:, cb, :],
                                     start=(cb == 0), stop=(cb == 1))
                dst = uP if s < 2 else uF
                nc.scalar.activation(dst[:, 1 + tt * 512: 513 + tt * 512,
                                         s % 2],
                                     ups[:, :], AF.Copy)

        def emit_G(i, tt):
            # gather P/F on gpsimd and combine. The source AP is a prefix
            # window covering exactly the reachable indices (<= 64 tokens
            # into tile tt+1): subtile deps then give RAW edges on just the
            # u-tiles needed and no WAR edge against later u-copies.
            W = min(NE, 512 * (tt + 1) + 65)
            gP = gpp.tile([128, 512, 2], BF16, tag="gP")
            nc.gpsimd.ap_gather(gP[:, :, :], uP[:, 0:W, :], ix_sb[i][:, tt, 0, :],
                                channels=128, num_elems=W, d=2, num_idxs=512)
            gF = gpp.tile([128, 512, 2], BF16, tag="gF")
            nc.gpsimd.ap_gather(gF[:, :, :], uF[:, 0:W, :], ix_sb[i][:, tt, 1, :],
                                channels=128, num_elems=W, d=2, num_idxs=512)
            # combine P+F per chunk plane on DVE (de-interleaves so the PE
            # inject streams a contiguous rhs). Keeping this off GPSIMD means
            # GPSIMD only ever runs ap_gather: no ucode library thrashing.
            pf = gpp.tile([128, 2, 512], BF16, tag="pf", bufs=4)
            for oc in range(2):
                nc.vector.tensor_tensor(pf[:, oc, :], gP[:, :, oc],
                                        gF[:, :, oc], OP.add)
            pf_tiles[tt] = pf

        def emit_V(tt, wct_sb):
            # v = lrelu(WC@xt + pf) -> v tile (channel-major, halo cols);
            # pf joins the PSUM group via a PE identity matmul
            v = vtp.tile([128, 2, 516], BF16, tag="v")
            v_tiles[tt] = v
            for oc in range(2):
                vps = psp.tile([128, 512], F32, tag=f"pv{oc}")
                for cb in range(2):
                    nc.tensor.matmul(vps[:, :], wct_sb[:, cb, oc, :],
                                     xt_tiles[tt][:, cb, :],
                                     start=(cb == 0), stop=False)
                nc.tensor.matmul(vps[:, :], ident_sb[:, :],
                                 pf_tiles[tt][:, oc, :],
                                 start=False, stop=True)
                nc.scalar.activation(v[:, oc, 1:513], vps[:, :],
                                     AF.Prelu, alpha=SLOPE)
            pf_tiles[tt] = None
            # halo stitch with previous tile
            if tt == 0:
                nc.vector.memset(v[:, :, 0:1], 0.0)
            else:
                nc.vector.tensor_copy(v[:, :, 0:1],
                                      v_tiles[tt - 1][:, :, 512:513])
                nc.vector.tensor_copy(v_tiles[tt - 1][:, :, 513:514],
                                      v[:, :, 1:2])
            if tt == nT - 1:
                nc.vector.memset(v[:, :, 513:514], 0.0)

        def emit_conv3(tt, wa_sb, b3_sb, last):
            tsl = bass.ts(tt, 512)
            for ob in range(2):
                yps = psp.tile([128, 512], F32, tag=f"pt{ob}")
                j = 0
                for k in range(3):
                    for cb in range(2):
                        nc.tensor.matmul(yps[:, :], wa_sb[:, k, cb, ob, :],
                                         v_tiles[tt][:, cb, k:k + 512],
                                         start=(j == 0), stop=(j == 5))
                        j += 1
                # residual: x = (y + b3) + x
                nc.vector.scalar_tensor_tensor(
                    x_sb[:, ob, tsl], yps[:, :], b3_sb[:, ob:ob + 1],
                    x_sb[:, ob, tsl], OP.add, OP.add)
            v_tiles[tt] = None
            if last:
                for cb in range(2):
                    nc.sync.dma_start(out_d[cb, :, tsl], x_sb[:, cb, tsl])

        # ---- pipelined schedule ----
        for i in range(NITER):
            wpf_sb, wct_sb, wa_sb, b3_sb = w_cur if i == 0 else w_next
            if i + 1 < NITER:
                w_next = load_weights(i + 1)
                load_ix(i + 1)
            if i > 0:
                xt_tiles, xt_next = xt_next, [None] * nT
            for k in range(nT + LAG_C):
                # gather first: its inputs are a full step old, so GPSIMD
                # starts immediately instead of chasing this step's ACT
                if LAG_G <= k < nT + LAG_G:
                    emit_G(i, k - LAG_G)
                if LAG_V <= k < nT + LAG_V:
                    emit_V(k - LAG_V, wct_sb)
                if k < nT:
                    if xt_tiles[k] is None:
                        xt_tiles[k] = emit_A(k)
                    emit_B(k, wpf_sb)
                if k >= LAG_C:
                    emit_conv3(k - LAG_C, wa_sb, b3_sb, i == NITER - 1)
            # warm the next iteration's first lrelu tiles (DVE runs them
            # under the tail conv3s so B(i+1, 0) starts immediately)
            if i + 1 < NITER:
                xt_next[0] = emit_A(0)
                xt_next[1] = emit_A(1)

    nc.compile()
    return nc


def _to_bf16(a):
    return np.asarray(a, dtype=np.float32).astype(ml_dtypes.bfloat16)


def prep_in_maps(x, d, WC, bC, WP, bP, WF, bF, WA, bA, T=T_FULL):
    """Build the 8 per-core input maps from the full-problem arrays."""
    x = np.asarray(x, dtype=np.float32)
    d = np.asarray(d, dtype=np.float32)
    WC, WP, WF, WA = (np.asarray(w, dtype=np.float32) for w in (WC, WP, WF, WA))
    bC, bP, bF, bA = (np.asarray(b, dtype=np.float32) for b in (bC, bP, bF, bA))
    nb = x.shape[0]
    nT = T // 512

    # weight layouts (see build_nc):
    #   wpf[i, p, cb, s, o]: u-channel s*128+o <- in-channel cb*128+p
    #     s in {0,1}: WP rows [0:128],[128:256]; s in {2,3}: WF rows.
    #   wct[i, p, cb, oc, o] = WC[i, oc*128+o, cb*128+p]
    #   wa[i, p, k, cb, ob, o] = WA[i, ob*128+o, cb*128+p, k]
    wpf = np.empty((NITER, 128, 2, 4, 128), np.float32)
    wct = np.empty((NITER, 128, 2, 2, 128), np.float32)
    wa = np.empty((NITER, 128, 3, 2, 2, 128), np.float32)
    for i in range(NITER):
        wpfT = np.concatenate([WP[i], WF[i]], axis=0)       # [512, 256]
        wpf[i] = wpfT.reshape(4, 128, 2, 128).transpose(3, 2, 0, 1)
        wct[i] = WC[i].reshape(2, 128, 2, 128).transpose(3, 2, 0, 1)
        wa[i] = WA[i].reshape(2, 128, 2, 128, 3).transpose(3, 4, 2, 0, 1)
    # bC/bP/bF are structurally zero in this problem; only bA is carried
    # (folded into the residual STT's per-partition scalar slot).
    b3 = bA.reshape(NITER, 2, 128).transpose(0, 2, 1).copy()
    wpf, wct, wa = _to_bf16(wpf), _to_bf16(wct), _to_bf16(wa)
    ident = _to_bf16(np.eye(128, dtype=np.float32))

    # gather indices, +1-shifted (0 = zero pad column), wrapped for ap_gather
    tf = np.arange(T, dtype=np.float32)
    in_maps = []
    for b in range(nb):
        dv = d[b, 0].astype(np.float32)
        ix = np.zeros((NITER, nT, 2, 128, 32), np.int16)
        for i, dil in enumerate(DILATIONS):
            dd = dv * np.float32(dil)
            rp = np.round(tf - dd).astype(np.int64)
            rf = np.round(tf + dd).astype(np.int64)
            rp = np.where(rp >= 0, rp + 1, 0)
            rf = np.where(rf < T, rf + 1, 0)
            for h, r in enumerate((rp, rf)):
                # [nT, 512] -> wrapped [nT, 32, 16] -> [nT, 16, 32], tiled x8
                w16 = r.reshape(nT, 32, 16).transpose(0, 2, 1)
                ix[i, :, h] = np.tile(w16, (1, 8, 1)).astype(np.int16)
        m = {
            "x": x[b].reshape(2, 128, T).copy(),
            "wpf": wpf, "wct": wct, "wa": wa,
            "b3": b3, "ix": ix, "ident": ident,
        }
        in_maps.append(m)
    return in_maps, True


_nc_cache = {}


def kernel(**inputs) -> np.ndarray:
    T = inputs["x"].shape[2]
    in_maps, has_b1 = prep_in_maps(**inputs, T=T)
    key = (T, has_b1)
    if key not in _nc_cache:
        _nc_cache[key] = build_nc(T)
    nc = _nc_cache[key]
    res = run_bass_kernel_spmd(nc, in_maps, core_ids=list(range(8)))
    out = np.stack([np.asarray(res.results[i]["out"], dtype=np.float32)
                    .reshape(C, T) for i in range(8)])
    return out


# revision 25
# speedup vs baseline: 3.3682x; 3.3682x over previous
"""Trainium2 Bass kernel for nn_AdaptiveResBlock (8-core data-parallel).

Reference computation (per batch element b, C=256 channels, T=8192 time):
  for i, dil in enumerate((1, 2, 4)):
      xt = lrelu(x)
      xP, xF = time-gather of xt at round(t -/+ d*dil), zero out-of-range
      xt = WC@xt + WP@xP + WF@xF + biases        (1x1 convs over channels)
      xt = lrelu(xt)
      xt = conv3(xt, WA) + bias
      x = xt + x

Structure used:
  * The time-gather commutes with the 1x1 convs (indices are per-time,
    shared across channels):  WP @ gather(xt) == gather(WP @ xt).
  * Gather offsets are bounded by max(d)*dil <= 64 < 128, so for output
    token block b (128 tokens) the P-gather sources live in token blocks
    {b-1, b} and the F-gather sources in {b, b+1}.
  * The gather itself runs on the TensorEngine as one-hot matmuls in
    token-major space: S[j, t] = (idx(t) == j) built on DVE via is_equal
    against an iota column (relative indices DMA-broadcast from DRAM);
    out-of-range indices get no one-hot bit => free zero masking.
  * Per token block the PSUM accumulation fuses: uC^T (xt-stationary
    matmuls) + one-hot gathers of uP^T / uF^T (+ optional bias rank-1
    matmul).  ACT applies leaky-relu (Prelu, exact on HW) straight from
    PSUM; PE transposes v^T back to channel-major for the 3-tap conv.
  * The u^T token stripe is a rolling window of per-block SBUF tiles
    (B-stage runs a few blocks ahead of the gather stage).
  * lrelu(x) for the next iteration is emitted right behind each tile's
    residual update so the next iteration's matmuls start immediately.

Sharded data-parallel over B=8 across the 8 NeuronCores; (C,C) weights
replicated; per-core relative-gather-index tensors precomputed from d.
"""

import numpy as np
import ml_dtypes
from contextlib import ExitStack

import concourse.bass as bass
import concourse.tile as tile
from concourse import mybir, bacc
from concourse.bass_utils import run_bass_kernel_spmd

F32 = mybir.dt.float32
BF16 = mybir.dt.bfloat16
I16 = mybir.dt.int16
AF = mybir.ActivationFunctionType
OP = mybir.AluOpType

B, C, T_FULL = 8, 256, 8192
DILATIONS = (1, 2, 4)
NITER = len(DILATIONS)
SLOPE = 0.1
INVALID = -512
LAG = 3  # B-stage leads the gather stage by this many token blocks


def build_nc(T=T_FULL, num_devices=8, has_b1=False):
    nT = T // 512            # 512-wide time tiles
    nR = T // 128            # 128-wide token blocks

    nc = bacc.Bacc("TRN2", target_bir_lowering=False, debug=False,
                   num_devices=num_devices)
    x_d = nc.declare_dram_parameter("x", [2, 128, T], F32, isOutput=False)
    wpf_d = nc.declare_dram_parameter("wpf", [NITER, 2, 128, 512], BF16,
                                      isOutput=False)
    wct_d = nc.declare_dram_parameter("wct", [NITER, 2, 128, 256], BF16,
                                      isOutput=False)
    wa_d = nc.declare_dram_parameter("wa", [NITER, 3, 2, 2, 128, 128], BF16,
                                     isOutput=False)
    b3_d = nc.declare_dram_parameter("b3", [NITER, 2, 128, 1], F32,
                                     isOutput=False)
    ixr_d = nc.declare_dram_parameter("ixrel", [NITER, nR, 512], mybir.dt.int8,
                                      isOutput=False)
    iota_d = nc.declare_dram_parameter("iota", [128, 1], F32, isOutput=False)
    id_d = nc.declare_dram_parameter("ident", [128, 128], BF16, isOutput=False)
    if has_b1:
        b1r_d = nc.declare_dram_parameter("b1row", [NITER, 1, 256], BF16,
                                          isOutput=False)
        ones_d = nc.declare_dram_parameter("ones", [1, 128], BF16,
                                           isOutput=False)
    out_d = nc.declare_dram_parameter("out", [2, 128, T], F32, isOutput=True)

    with tile.TileContext(nc) as tc, ExitStack() as ctx:
        xpool = ctx.enter_context(tc.tile_pool(name="xres", bufs=1))
        stp = ctx.enter_context(tc.tile_pool(name="stp", bufs=LAG + 4))
        xtp = ctx.enter_context(tc.tile_pool(name="xtp", bufs=nT))
        vp = ctx.enter_context(tc.tile_pool(name="vp", bufs=nT))
        vtp = ctx.enter_context(tc.tile_pool(name="vtp", bufs=4))
        relp = ctx.enter_context(tc.tile_pool(name="relp", bufs=6))
        sp = ctx.enter_context(tc.tile_pool(name="sp", bufs=6))
        wts = ctx.enter_context(tc.tile_pool(name="wts", bufs=2))
        cst = ctx.enter_context(tc.tile_pool(name="cst", bufs=1))
        pu_ps = ctx.enter_context(tc.tile_pool(name="pu", bufs=2, space="PSUM"))
        pv_ps = ctx.enter_context(tc.tile_pool(name="pv", bufs=2, space="PSUM"))
        pt_ps = ctx.enter_context(tc.tile_pool(name="pt", bufs=4, space="PSUM"))

        def load_weights(i):
            # consolidated single-DMA loads on the ACT ring: keeps the sync
            # ring free for the x stream and avoids per-DMA DGE setup cost
            wpf_sb = wts.tile([128, 2, 512], BF16, tag="wpf")
            nc.scalar.dma_start(wpf_sb[:, :, :],
                                wpf_d[i].rearrange("c p n -> p c n"))
            wct_sb = wts.tile([128, 2, 256], BF16, tag="wct")
            nc.scalar.dma_start(wct_sb[:, :, :],
                                wct_d[i].rearrange("c p n -> p c n"))
            wa_sb = wts.tile([128, 3, 2, 2, 128], BF16, tag="wa")
            nc.scalar.dma_start(wa_sb[:, :, :, :, :],
                                wa_d[i].rearrange("k c o p f -> p k c o f"))
            b3_sb = wts.tile([128, 2], F32, tag="b3")
            nc.scalar.dma_start(b3_sb[:, :],
                                b3_d[i].rearrange("o p u -> p (o u)"))
            b1r_sb = None
            if has_b1:
                b1r_sb = wts.tile([1, 256], BF16, tag="b1r")
                nc.scalar.dma_start(b1r_sb[:, :], b1r_d[i])
            return wpf_sb, wct_sb, wa_sb, b3_sb, b1r_sb

        # Head staging: the first columns of x land in a small dedicated
        # tile with an unambiguous early dependency, so iteration 0 starts
        # without waiting for the bulk x load. Small pieces so the very
        # first tile arrives within a couple of microseconds.
        HEAD = min(2048, T)
        x_head = cst.tile([128, 2, HEAD], F32)
        for sg in range(HEAD // 512):
            sl = bass.ts(sg, 512)
            nc.sync.dma_start(x_head[:, :, sl],
                              x_d[:, :, sl].rearrange("c p n -> p c n"))
        w0 = load_weights(0)
        iota_sb = cst.tile([128, 1], F32)
        nc.scalar.dma_start(iota_sb[:, :], iota_d[:, :])
        ident_sb = cst.tile([128, 128], BF16)
        nc.scalar.dma_start(ident_sb[:, :], id_d[:, :])
        if has_b1:
            ones_sb = cst.tile([1, 128], BF16)
            nc.scalar.dma_start(ones_sb[:, :], ones_d[:, :])

        # Resident fp32 signal.  Columns [0, HEAD) are served by x_head in
        # iteration 0 and fully written by its residuals, so skip them here.
        # The bulk load runs on the gpsimd+vector rings, keeping the sync
        # queue free for the rel broadcasts that gate the first S builds.
        x_sb = xpool.tile([128, 2, T], F32)
        for sg in range((T - HEAD) // 512):
            sl = bass.ds(HEAD + sg * 512, 512)
            for cb in range(2):
                nc.gpsimd.dma_start(x_sb[:, cb, sl], x_d[cb, :, sl])

        st_tiles = [None] * nR   # rolling u^T stripe tiles
        xt_tiles = [None] * nT
        xt_next = [None] * nT
        v_tiles = [None] * nT
        vt_tiles = [None] * (nR // 2)
        pv_tiles = [None] * (nR // 2)

        def emit_A(tt, head=False):
            tsl = bass.ts(tt, 512)
            src = x_head[:, :, tsl] if head else x_sb[:, :, tsl]
            xt = xtp.tile([128, 2, 512], BF16, tag="xt")
            nc.vector.scalar_tensor_tensor(
                xt[:, :, :], src, SLOPE, src, OP.mult, OP.max)
            return xt

        def emit_conv3(tt, wa_sb, b3_sb, last, head=False):
            tsl = bass.ts(tt, 512)
            for ob in range(2):
                py = pt_ps.tile([128, 512], F32, tag="pt")
                j = 0
                for k in range(3):
                    for cb in range(2):
                        nc.tensor.matmul(py[:, :], wa_sb[:, k, cb, ob, :],
                                         v_tiles[tt][:, cb, k:k + 512],
                                         start=(j == 0), stop=(j == 5))
                        j += 1
                # residual:  x = (y + b3) + x  (iter-0 head tiles read the
                # staging copy, write the resident tensor)
                xin = x_head[:, ob, tsl] if head else x_sb[:, ob, tsl]
                nc.vector.scalar_tensor_tensor(
                    x_sb[:, ob, tsl], py[:, :], b3_sb[:, ob:ob + 1],
                    xin, OP.add, OP.add)
            if last:
                for cb in range(2):
                    nc.sync.dma_start(out_d[cb, :, tsl], x_sb[:, cb, tsl])

        for i in range(NITER):
            wpf_sb, wct_sb, wa_sb, b3_sb, b1r_sb = \
                w0 if i == 0 else load_weights(i)

            if i > 0:
                xt_tiles, xt_next = xt_next, [None] * nT
            # iteration-0 lrelus are emitted just-in-time inside the block
            # loop so they don't hog the DVE queue ahead of the S builds

            def emit_B(b):
                tt, off = b // 4, (b % 4) * 128
                ps = pu_ps.tile([128, 512], F32, tag="pu")
                nc.tensor.matmul(ps[:, :], xt_tiles[tt][:, 0, off:off + 128],
                                 wpf_sb[:, 0, :], start=True, stop=False)
                nc.tensor.matmul(ps[:, :], xt_tiles[tt][:, 1, off:off + 128],
                                 wpf_sb[:, 1, :], start=False, stop=True)
                st = stp.tile([128, 512], BF16, tag="st")
                st_tiles[b] = st
                nc.scalar.activation(st[:, :], ps[:, :], AF.Copy)

            def emit_G(b):
                tt, off = b // 4, (b % 4) * 128
                rel = relp.tile([128, 512], mybir.dt.int8, tag="rel")
                nc.sync.dma_start(rel[:, :],
                                  ixr_d[i, b].partition_broadcast(128))
                S = sp.tile([128, 512], BF16, tag="S")
                nc.vector.tensor_scalar(S[:, :], rel[:, :], iota_sb[:, 0:1],
                                        None, OP.is_equal)
                pr = b // 2
                half = (b % 2) * 256
                if b % 2 == 0:
                    pv_tile = pv_ps.tile([128, 512], F32, tag="pv")
                    pv_tiles[pr] = pv_tile
                pv = pv_tiles[pr]
                out_sl = pv[:, half:half + 256]
                mms = [
                    (xt_tiles[tt][:, 0, off:off + 128], wct_sb[:, 0, :]),
                    (xt_tiles[tt][:, 1, off:off + 128], wct_sb[:, 1, :]),
                ]
                if has_b1:
                    mms.append((ones_sb[:, :], b1r_sb[:, :]))
                if b > 0:
                    mms.append((S[:, 0:128], st_tiles[b - 1][:, 0:256]))
                mms.append((S[:, 128:256], st_tiles[b][:, 0:256]))
                mms.append((S[:, 256:384], st_tiles[b][:, 256:512]))
                if b < nR - 1:
                    mms.append((S[:, 384:512], st_tiles[b + 1][:, 256:512]))
                for j, (lhsT, rhs) in enumerate(mms):
                    nc.tensor.matmul(out_sl, lhsT, rhs, start=(j == 0),
                                     stop=(j == len(mms) - 1))
                if b % 2 == 1:
                    vt = vtp.tile([128, 512], BF16, tag="vt")
                    vt_tiles[pr] = vt
                    nc.scalar.activation(vt[:, :], pv[:, :], AF.Prelu,
                                         alpha=SLOPE)
                if b % 4 == 3:
                    emit_quad(b // 4)

            def emit_quad(q):
                # transpose blocks 4q..4q+3 back to channel-major v tile
                v = vp.tile([128, 2, 516], BF16, tag="v")
                v_tiles[q] = v
                for cb in range(2):
                    pt = pt_ps.tile([128, 512], BF16, tag="pt")
                    for j4 in range(4):
                        vt = vt_tiles[2 * q + j4 // 2]
                        csl = (j4 % 2) * 256 + cb * 128
                        nc.tensor.transpose(pt[:, j4 * 128:(j4 + 1) * 128],
                                            vt[:, csl:csl + 128],
                                            ident_sb[:, :])
                    nc.scalar.activation(v[:, cb, 1:513], pt[:, :], AF.Copy)
                if q == 0:
                    nc.vector.memset(v[:, :, 0:1], 0.0)
                else:
                    nc.vector.tensor_copy(v[:, :, 0:1],
                                          v_tiles[q - 1][:, :, 512:513])
                    nc.vector.tensor_copy(v_tiles[q - 1][:, :, 513:514],
                                          v[:, :, 1:2])
                if q == nT - 1:
                    nc.vector.memset(v[:, :, 513:514], 0.0)
                if q >= 1:
                    emit_conv3(q - 1, wa_sb, b3_sb, i == NITER - 1,
                               head=(i == 0 and (q - 1) * 512 < HEAD))
                    if i + 1 < NITER:
                        xt_next[q - 1] = emit_A(q - 1)

            for b in range(nR + LAG):
                if b < nR:
                    tt = b // 4
                    if xt_tiles[tt] is None:
                        xt_tiles[tt] = emit_A(tt, head=(tt * 512 < HEAD))
                    emit_B(b)
                if b >= LAG:
                    emit_G(b - LAG)
            emit_conv3(nT - 1, wa_sb, b3_sb, i == NITER - 1,
                       head=(i == 0 and (nT - 1) * 512 < HEAD))
            if i + 1 < NITER:
                xt_next[nT - 1] = emit_A(nT - 1)

    nc.compile()
    return nc


def _to_bf16(a):
    return np.asarray(a, dtype=np.float32).astype(ml_dtypes.bfloat16)


def prep_in_maps(x, d, WC, bC, WP, bP, WF, bF, WA, bA, T=T_FULL):
    """Build the 8 per-core input maps from the full-problem arrays.
    Returns (in_maps, has_b1)."""
    x = np.asarray(x, dtype=np.float32)
    d = np.asarray(d, dtype=np.float32)
    WC, WP, WF, WA = (np.asarray(w, dtype=np.float32) for w in (WC, WP, WF, WA))
    bC, bP, bF, bA = (np.asarray(b, dtype=np.float32) for b in (bC, bP, bF, bA))
    nb = x.shape[0]
    nR = T // 128

    wpf = np.empty((NITER, 2, 128, 512), np.float32)
    wct = np.empty((NITER, 2, 128, 256), np.float32)
    wa = np.empty((NITER, 3, 2, 2, 128, 128), np.float32)
    for i in range(NITER):
        wpfT = np.concatenate([WP[i].T, WF[i].T], axis=1)  # [c', 512]
        wpf[i] = wpfT.reshape(2, 128, 512)
        wct[i] = WC[i].T.reshape(2, 128, 256)              # [cc, p, o]
        for k in range(3):
            waT = WA[i, :, :, k].T                         # [c', o]
            wa[i, k] = waT.reshape(2, 128, 2, 128).transpose(0, 2, 1, 3)
    b1 = (bC + bP + bF).astype(np.float32)                  # [NITER, 256]
    has_b1 = bool(np.any(b1 != 0))
    b3 = bA.reshape(NITER, 2, 128, 1).astype(np.float32)

    wpf, wct, wa = _to_bf16(wpf), _to_bf16(wct), _to_bf16(wa)
    iota = np.arange(128, dtype=np.float32).reshape(128, 1)
    ident = _to_bf16(np.eye(128, dtype=np.float32))

    tf = np.arange(T, dtype=np.float32)
    in_maps = []
    for b in range(nb):
        dv = d[b, 0].astype(np.float32)
        ixr = np.full((NITER, nR, 512), INVALID, np.int16)
        for i, dil in enumerate(DILATIONS):
            dd = dv * np.float32(dil)
            rp = np.round(tf - dd).astype(np.int64)
            rf = np.round(tf + dd).astype(np.int64)
            rp = np.where(rp >= 0, rp, np.int64(-(1 << 32)))
            rf = np.where(rf < T, rf, np.int64(1 << 32))
            blk = np.arange(nR).repeat(128) * 128           # (T,)
            for c, base in enumerate((rp - blk + 128, rp - blk,
                                      rf - blk, rf - blk - 128)):
                v = np.where((base >= 0) & (base < 128), base,
                             np.int64(INVALID)).astype(np.int16)
                ixr[i, :, c * 128:(c + 1) * 128] = v.reshape(nR, 128)
        m = {
            "x": x[b].reshape(2, 128, T).copy(),
            "wpf": wpf, "wct": wct, "wa": wa, "b3": b3,
            "ixrel": np.clip(ixr, -128, 127).astype(np.int8),
            "iota": iota, "ident": ident,
        }
        if has_b1:
            m["b1row"] = _to_bf16(b1.reshape(NITER, 1, 256))
            m["ones"] = _to_bf16(np.ones((1, 128), np.float32))
        in_maps.append(m)
    return in_maps, has_b1


_nc_cache = {}


def kernel(**inputs) -> np.ndarray:
    T = inputs["x"].shape[2]
    in_maps, has_b1 = prep_in_maps(**inputs, T=T)
    key = (T, has_b1)
    if key not in _nc_cache:
        _nc_cache[key] = build_nc(T, has_b1=has_b1)
    nc = _nc_cache[key]
    res = run_bass_kernel_spmd(nc, in_maps, core_ids=list(range(8)))
    out = np.stack([np.asarray(res.results[i]["out"], dtype=np.float32)
                    .reshape(C, T) for i in range(8)])
    return out



# revision 27
# speedup vs baseline: 3.5906x; 1.0660x over previous
"""Trainium2 Bass kernel for nn_AdaptiveResBlock (8-core data-parallel).

Reference computation (per batch element b, C=256 channels, T=8192 time):
  for i, dil in enumerate((1, 2, 4)):
      xt = lrelu(x)
      xP, xF = time-gather of xt at round(t -/+ d*dil), zero out-of-range
      xt = WC@xt + WP@xP + WF@xF + biases        (1x1 convs over channels)
      xt = lrelu(xt)
      xt = conv3(xt, WA) + bias
      x = xt + x

Structure used:
  * The time-gather commutes with the 1x1 convs (indices are per-time,
    shared across channels):  WP @ gather(xt) == gather(WP @ xt).
  * Gather offsets are bounded by max(d)*dil <= 64 < 128, so for output
    token block b (128 tokens) the P-gather sources live in token blocks
    {b-1, b} and the F-gather sources in {b, b+1}.
  * The gather itself runs on the TensorEngine as one-hot matmuls in
    token-major space: S[j, t] = (idx(t) == j) built on DVE via is_equal
    against an iota column (relative indices DMA-broadcast from DRAM);
    out-of-range indices get no one-hot bit => free zero masking.
  * Per token block the PSUM accumulation fuses: uC^T (xt-stationary
    matmuls) + one-hot gathers of uP^T / uF^T (+ optional bias rank-1
    matmul).  ACT applies leaky-relu (Prelu, exact on HW) straight from
    PSUM; PE transposes v^T back to channel-major for the 3-tap conv.
  * The u^T token stripe is a rolling window of per-block SBUF tiles
    (B-stage runs a few blocks ahead of the gather stage).
  * lrelu(x) for the next iteration is emitted right behind each tile's
    residual update so the next iteration's matmuls start immediately.

Sharded data-parallel over B=8 across the 8 NeuronCores; (C,C) weights
replicated; per-core relative-gather-index tensors precomputed from d.
"""

import numpy as np
import ml_dtypes
from contextlib import ExitStack

import concourse.bass as bass
import concourse.tile as tile
from concourse import mybir, bacc
from concourse.bass_utils import run_bass_kernel_spmd

F32 = mybir.dt.float32
BF16 = mybir.dt.bfloat16
I16 = mybir.dt.int16
AF = mybir.ActivationFunctionType
OP = mybir.AluOpType

B, C, T_FULL = 8, 256, 8192
DILATIONS = (1, 2, 4)
NITER = len(DILATIONS)
SLOPE = 0.1
INVALID = -512
LAG = 3  # B-stage leads the gather stage by this many token blocks


def build_nc(T=T_FULL, num_devices=8, has_b1=False):
    nT = T // 512            # 512-wide time tiles
    nR = T // 128            # 128-wide token blocks

    nc = bacc.Bacc("TRN2", target_bir_lowering=False, debug=False,
                   num_devices=num_devices)
    x_d = nc.declare_dram_parameter("x", [2, 128, T], F32, isOutput=False)
    wpf_d = nc.declare_dram_parameter("wpf", [NITER, 2, 128, 512], BF16,
                                      isOutput=False)
    wct_d = nc.declare_dram_parameter("wct", [NITER, 2, 128, 256], BF16,
                                      isOutput=False)
    wa_d = nc.declare_dram_parameter("wa", [NITER, 3, 2, 2, 128, 128], BF16,
                                     isOutput=False)
    b3_d = nc.declare_dram_parameter("b3", [NITER, 2, 128, 1], F32,
                                     isOutput=False)
    ixr_d = nc.declare_dram_parameter("ixrel", [NITER, nR, 512], mybir.dt.int8,
                                      isOutput=False)
    iota_d = nc.declare_dram_parameter("iota", [128, 1], F32, isOutput=False)
    id_d = nc.declare_dram_parameter("ident", [128, 128], BF16, isOutput=False)
    if has_b1:
        b1r_d = nc.declare_dram_parameter("b1row", [NITER, 1, 256], BF16,
                                          isOutput=False)
        ones_d = nc.declare_dram_parameter("ones", [1, 128], BF16,
                                           isOutput=False)
    out_d = nc.declare_dram_parameter("out", [2, 128, T], F32, isOutput=True)

    with tile.TileContext(nc) as tc, ExitStack() as ctx:
        xpool = ctx.enter_context(tc.tile_pool(name="xres", bufs=1))
        stp = ctx.enter_context(tc.tile_pool(name="stp", bufs=LAG + 4))
        xtp = ctx.enter_context(tc.tile_pool(name="xtp", bufs=nT))
        vp = ctx.enter_context(tc.tile_pool(name="vp", bufs=nT))
        vtp = ctx.enter_context(tc.tile_pool(name="vtp", bufs=4))
        relp = ctx.enter_context(tc.tile_pool(name="relp", bufs=6))
        sp = ctx.enter_context(tc.tile_pool(name="sp", bufs=6))
        wts = ctx.enter_context(tc.tile_pool(name="wts", bufs=2))
        cst = ctx.enter_context(tc.tile_pool(name="cst", bufs=1))
        pu_ps = ctx.enter_context(tc.tile_pool(name="pu", bufs=2, space="PSUM"))
        pv_ps = ctx.enter_context(tc.tile_pool(name="pv", bufs=2, space="PSUM"))
        pt_ps = ctx.enter_context(tc.tile_pool(name="pt", bufs=4, space="PSUM"))

        def load_weights(i):
            # consolidated single-DMA loads on the ACT ring: keeps the sync
            # ring free for the x stream and avoids per-DMA DGE setup cost
            wpf_sb = wts.tile([128, 2, 512], BF16, tag="wpf")
            nc.scalar.dma_start(wpf_sb[:, :, :],
                                wpf_d[i].rearrange("c p n -> p c n"))
            wct_sb = wts.tile([128, 2, 256], BF16, tag="wct")
            nc.scalar.dma_start(wct_sb[:, :, :],
                                wct_d[i].rearrange("c p n -> p c n"))
            wa_sb = wts.tile([128, 3, 2, 2, 128], BF16, tag="wa")
            nc.scalar.dma_start(wa_sb[:, :, :, :, :],
                                wa_d[i].rearrange("k c o p f -> p k c o f"))
            b3_sb = wts.tile([128, 2], F32, tag="b3")
            nc.scalar.dma_start(b3_sb[:, :],
                                b3_d[i].rearrange("o p u -> p (o u)"))
            b1r_sb = None
            if has_b1:
                b1r_sb = wts.tile([1, 256], BF16, tag="b1r")
                nc.scalar.dma_start(b1r_sb[:, :], b1r_d[i])
            return wpf_sb, wct_sb, wa_sb, b3_sb, b1r_sb

        # Head staging: the first columns of x land in a small dedicated
        # tile with an unambiguous early dependency, so iteration 0 starts
        # without waiting for the bulk x load. Small pieces so the very
        # first tile arrives within a couple of microseconds.
        HEAD = min(2048, T)
        x_head = cst.tile([128, 2, HEAD], F32)
        for sg in range(HEAD // 512):
            sl = bass.ts(sg, 512)
            nc.sync.dma_start(x_head[:, :, sl],
                              x_d[:, :, sl].rearrange("c p n -> p c n"))
        w0 = load_weights(0)
        iota_sb = cst.tile([128, 1], F32)
        nc.scalar.dma_start(iota_sb[:, :], iota_d[:, :])
        ident_sb = cst.tile([128, 128], BF16)
        nc.scalar.dma_start(ident_sb[:, :], id_d[:, :])
        if has_b1:
            ones_sb = cst.tile([1, 128], BF16)
            nc.scalar.dma_start(ones_sb[:, :], ones_d[:, :])

        # Resident fp32 signal.  Columns [0, HEAD) are served by x_head in
        # iteration 0 and fully written by its residuals, so skip them here.
        # The bulk load runs on the gpsimd+vector rings, keeping the sync
        # queue free for the rel broadcasts that gate the first S builds.
        x_sb = xpool.tile([128, 2, T], F32)
        for sg in range((T - HEAD) // 512):
            sl = bass.ds(HEAD + sg * 512, 512)
            for cb in range(2):
                nc.gpsimd.dma_start(x_sb[:, cb, sl], x_d[cb, :, sl])

        st_tiles = [None] * nR   # rolling u^T stripe tiles
        xt_tiles = [None] * nT
        xt_next = [None] * nT
        v_tiles = [None] * nT
        vt_tiles = [None] * (nR // 2)
        pv_tiles = [None] * (nR // 2)

        def emit_A(tt, head=False):
            tsl = bass.ts(tt, 512)
            src = x_head[:, :, tsl] if head else x_sb[:, :, tsl]
            xt = xtp.tile([128, 2, 512], BF16, tag="xt")
            nc.vector.scalar_tensor_tensor(
                xt[:, :, :], src, SLOPE, src, OP.mult, OP.max)
            return xt

        def emit_conv3(tt, wa_sb, b3_sb, last, head=False):
            tsl = bass.ts(tt, 512)
            for ob in range(2):
                py = pt_ps.tile([128, 512], F32, tag="pt")
                j = 0
                for k in range(3):
                    for cb in range(2):
                        nc.tensor.matmul(py[:, :], wa_sb[:, k, cb, ob, :],
                                         v_tiles[tt][:, cb, k:k + 512],
                                         start=(j == 0), stop=(j == 5))
                        j += 1
                # residual:  x = (y + b3) + x  (iter-0 head tiles read the
                # staging copy, write the resident tensor)
                xin = x_head[:, ob, tsl] if head else x_sb[:, ob, tsl]
                nc.vector.scalar_tensor_tensor(
                    x_sb[:, ob, tsl], py[:, :], b3_sb[:, ob:ob + 1],
                    xin, OP.add, OP.add)
            if last:
                for cb in range(2):
                    nc.sync.dma_start(out_d[cb, :, tsl], x_sb[:, cb, tsl])

        for i in range(NITER):
            wpf_sb, wct_sb, wa_sb, b3_sb, b1r_sb = \
                w0 if i == 0 else load_weights(i)

            if i > 0:
                xt_tiles, xt_next = xt_next, [None] * nT
            else:
                # iteration-0 lrelus run two block-groups ahead of use (not
                # all upfront: 16 queued lrelus would delay the S builds that
                # gate the gather matmuls by ~20us)
                xt_tiles[0] = emit_A(0, head=True)
                xt_tiles[1] = emit_A(1, head=True)

            def emit_B(b):
                tt, off = b // 4, (b % 4) * 128
                ps = pu_ps.tile([128, 512], F32, tag="pu")
                nc.tensor.matmul(ps[:, :], xt_tiles[tt][:, 0, off:off + 128],
                                 wpf_sb[:, 0, :], start=True, stop=False)
                nc.tensor.matmul(ps[:, :], xt_tiles[tt][:, 1, off:off + 128],
                                 wpf_sb[:, 1, :], start=False, stop=True)
                st = stp.tile([128, 512], BF16, tag="st")
                st_tiles[b] = st
                nc.scalar.activation(st[:, :], ps[:, :], AF.Copy)

            def emit_G(b):
                tt, off = b // 4, (b % 4) * 128
                rel = relp.tile([128, 512], mybir.dt.int8, tag="rel")
                nc.sync.dma_start(rel[:, :],
                                  ixr_d[i, b].partition_broadcast(128))
                S = sp.tile([128, 512], BF16, tag="S")
                nc.vector.tensor_scalar(S[:, :], rel[:, :], iota_sb[:, 0:1],
                                        None, OP.is_equal)
                pr = b // 2
                half = (b % 2) * 256
                if b % 2 == 0:
                    pv_tile = pv_ps.tile([128, 512], F32, tag="pv")
                    pv_tiles[pr] = pv_tile
                pv = pv_tiles[pr]
                out_sl = pv[:, half:half + 256]
                mms = [
                    (xt_tiles[tt][:, 0, off:off + 128], wct_sb[:, 0, :]),
                    (xt_tiles[tt][:, 1, off:off + 128], wct_sb[:, 1, :]),
                ]
                if has_b1:
                    mms.append((ones_sb[:, :], b1r_sb[:, :]))
                if b > 0:
                    mms.append((S[:, 0:128], st_tiles[b - 1][:, 0:256]))
                mms.append((S[:, 128:256], st_tiles[b][:, 0:256]))
                mms.append((S[:, 256:384], st_tiles[b][:, 256:512]))
                if b < nR - 1:
                    mms.append((S[:, 384:512], st_tiles[b + 1][:, 256:512]))
                for j, (lhsT, rhs) in enumerate(mms):
                    nc.tensor.matmul(out_sl, lhsT, rhs, start=(j == 0),
                                     stop=(j == len(mms) - 1))
                if b % 2 == 1:
                    vt = vtp.tile([128, 512], BF16, tag="vt")
                    vt_tiles[pr] = vt
                    nc.scalar.activation(vt[:, :], pv[:, :], AF.Prelu,
                                         alpha=SLOPE)
                if b % 4 == 3:
                    emit_quad(b // 4)

            def emit_quad(q):
                # transpose blocks 4q..4q+3 back to channel-major v tile
                v = vp.tile([128, 2, 516], BF16, tag="v")
                v_tiles[q] = v
                for cb in range(2):
                    pt = pt_ps.tile([128, 512], BF16, tag="pt")
                    for j4 in range(4):
                        vt = vt_tiles[2 * q + j4 // 2]
                        csl = (j4 % 2) * 256 + cb * 128
                        nc.tensor.transpose(pt[:, j4 * 128:(j4 + 1) * 128],
                                            vt[:, csl:csl + 128],
                                            ident_sb[:, :])
                    nc.scalar.activation(v[:, cb, 1:513], pt[:, :], AF.Copy)
                if q == 0:
                    nc.vector.memset(v[:, :, 0:1], 0.0)
                else:
                    nc.vector.tensor_copy(v[:, :, 0:1],
                                          v_tiles[q - 1][:, :, 512:513])
                    nc.vector.tensor_copy(v_tiles[q - 1][:, :, 513:514],
                                          v[:, :, 1:2])
                if q == nT - 1:
                    nc.vector.memset(v[:, :, 513:514], 0.0)
                if q >= 1:
                    emit_conv3(q - 1, wa_sb, b3_sb, i == NITER - 1,
                               head=(i == 0 and (q - 1) * 512 < HEAD))
                    if i + 1 < NITER:
                        xt_next[q - 1] = emit_A(q - 1)

            for b in range(nR + LAG):
                if b < nR:
                    if b % 4 == 0:
                        t2 = b // 4 + 2
                        if t2 < nT and xt_tiles[t2] is None:
                            xt_tiles[t2] = emit_A(t2, head=(t2 * 512 < HEAD))
                    emit_B(b)
                if b >= LAG:
                    emit_G(b - LAG)
            emit_conv3(nT - 1, wa_sb, b3_sb, i == NITER - 1,
                       head=(i == 0 and (nT - 1) * 512 < HEAD))
            if i + 1 < NITER:
                xt_next[nT - 1] = emit_A(nT - 1)

    nc.compile()
    return nc


def _to_bf16(a):
    return np.asarray(a, dtype=np.float32).astype(ml_dtypes.bfloat16)


def prep_in_maps(x, d, WC, bC, WP, bP, WF, bF, WA, bA, T=T_FULL):
    """Build the 8 per-core input maps from the full-problem arrays.
    Returns (in_maps, has_b1)."""
    x = np.asarray(x, dtype=np.float32)
    d = np.asarray(d, dtype=np.float32)
    WC, WP, WF, WA = (np.asarray(w, dtype=np.float32) for w in (WC, WP, WF, WA))
    bC, bP, bF, bA = (np.asarray(b, dtype=np.float32) for b in (bC, bP, bF, bA))
    nb = x.shape[0]
    nR = T // 128

    wpf = np.empty((NITER, 2, 128, 512), np.float32)
    wct = np.empty((NITER, 2, 128, 256), np.float32)
    wa = np.empty((NITER, 3, 2, 2, 128, 128), np.float32)
    for i in range(NITER):
        wpfT = np.concatenate([WP[i].T, WF[i].T], axis=1)  # [c', 512]
        wpf[i] = wpfT.reshape(2, 128, 512)
        wct[i] = WC[i].T.reshape(2, 128, 256)              # [cc, p, o]
        for k in range(3):
            waT = WA[i, :, :, k].T                         # [c', o]
            wa[i, k] = waT.reshape(2, 128, 2, 128).transpose(0, 2, 1, 3)
    b1 = (bC + bP + bF).astype(np.float32)                  # [NITER, 256]
    has_b1 = bool(np.any(b1 != 0))
    b3 = bA.reshape(NITER, 2, 128, 1).astype(np.float32)

    wpf, wct, wa = _to_bf16(wpf), _to_bf16(wct), _to_bf16(wa)
    iota = np.arange(128, dtype=np.float32).reshape(128, 1)
    ident = _to_bf16(np.eye(128, dtype=np.float32))

    tf = np.arange(T, dtype=np.float32)
    in_maps = []
    for b in range(nb):
        dv = d[b, 0].astype(np.float32)
        ixr = np.full((NITER, nR, 512), INVALID, np.int16)
        for i, dil in enumerate(DILATIONS):
            dd = dv * np.float32(dil)
            rp = np.round(tf - dd).astype(np.int64)
            rf = np.round(tf + dd).astype(np.int64)
            rp = np.where(rp >= 0, rp, np.int64(-(1 << 32)))
            rf = np.where(rf < T, rf, np.int64(1 << 32))
            blk = np.arange(nR).repeat(128) * 128           # (T,)
            for c, base in enumerate((rp - blk + 128, rp - blk,
                                      rf - blk, rf - blk - 128)):
                v = np.where((base >= 0) & (base < 128), base,
                             np.int64(INVALID)).astype(np.int16)
                ixr[i, :, c * 128:(c + 1) * 128] = v.reshape(nR, 128)
        m = {
            "x": x[b].reshape(2, 128, T).copy(),
            "wpf": wpf, "wct": wct, "wa": wa, "b3": b3,
            "ixrel": np.clip(ixr, -128, 127).astype(np.int8),
            "iota": iota, "ident": ident,
        }
        if has_b1:
            m["b1row"] = _to_bf16(b1.reshape(NITER, 1, 256))
            m["ones"] = _to_bf16(np.ones((1, 128), np.float32))
        in_maps.append(m)
    return in_maps, has_b1


_nc_cache = {}


def kernel(**inputs) -> np.ndarray:
    T = inputs["x"].shape[2]
    in_maps, has_b1 = prep_in_maps(**inputs, T=T)
    key = (T, has_b1)
    if key not in _nc_cache:
        _nc_cache[key] = build_nc(T, has_b1=has_b1)
    nc = _nc_cache[key]
    res = run_bass_kernel_spmd(nc, in_maps, core_ids=list(range(8)))
    out = np.stack([np.asarray(res.results[i]["out"], dtype=np.float32)
                    .reshape(C, T) for i in range(8)])
    return out



# revision 28
# speedup vs baseline: 3.6187x; 1.0078x over previous
"""Trainium2 Bass kernel for nn_AdaptiveResBlock (8-core data-parallel).

Reference computation (per batch element b, C=256 channels, T=8192 time):
  for i, dil in enumerate((1, 2, 4)):
      xt = lrelu(x)
      xP, xF = time-gather of xt at round(t -/+ d*dil), zero out-of-range
      xt = WC@xt + WP@xP + WF@xF + biases        (1x1 convs over channels)
      xt = lrelu(xt)
      xt = conv3(xt, WA) + bias
      x = xt + x

Structure used:
  * The time-gather commutes with the 1x1 convs (indices are per-time,
    shared across channels):  WP @ gather(xt) == gather(WP @ xt).
  * Gather offsets are bounded by max(d)*dil <= 64 < 128, so for output
    token block b (128 tokens) the P-gather sources live in token blocks
    {b-1, b} and the F-gather sources in {b, b+1}.
  * The gather itself runs on the TensorEngine as one-hot matmuls in
    token-major space: S[j, t] = (idx(t) == j) built on DVE via is_equal
    against an iota column (relative indices DMA-broadcast from DRAM);
    out-of-range indices get no one-hot bit => free zero masking.
  * Per token block the PSUM accumulation fuses: uC^T (xt-stationary
    matmuls) + one-hot gathers of uP^T / uF^T (+ optional bias rank-1
    matmul).  ACT applies leaky-relu (Prelu, exact on HW) straight from
    PSUM; PE transposes v^T back to channel-major for the 3-tap conv.
  * The u^T token stripe is a rolling window of per-block SBUF tiles
    (B-stage runs a few blocks ahead of the gather stage).
  * lrelu(x) for the next iteration is emitted right behind each tile's
    residual update so the next iteration's matmuls start immediately.

Sharded data-parallel over B=8 across the 8 NeuronCores; (C,C) weights
replicated; per-core relative-gather-index tensors precomputed from d.
"""

import numpy as np
import ml_dtypes
from contextlib import ExitStack

import concourse.bass as bass
import concourse.tile as tile
from concourse import mybir, bacc
from concourse.bass_utils import run_bass_kernel_spmd

F32 = mybir.dt.float32
BF16 = mybir.dt.bfloat16
I16 = mybir.dt.int16
AF = mybir.ActivationFunctionType
OP = mybir.AluOpType

B, C, T_FULL = 8, 256, 8192
DILATIONS = (1, 2, 4)
NITER = len(DILATIONS)
SLOPE = 0.1
INVALID = -512
LAG = 3  # B-stage leads the gather stage by this many token blocks


def build_nc(T=T_FULL, num_devices=8, has_b1=False):
    nT = T // 512            # 512-wide time tiles
    nR = T // 128            # 128-wide token blocks

    nc = bacc.Bacc("TRN2", target_bir_lowering=False, debug=False,
                   num_devices=num_devices)
    x_d = nc.declare_dram_parameter("x", [2, 128, T], F32, isOutput=False)
    wpf_d = nc.declare_dram_parameter("wpf", [NITER, 2, 128, 512], BF16,
                                      isOutput=False)
    wct_d = nc.declare_dram_parameter("wct", [NITER, 2, 128, 256], BF16,
                                      isOutput=False)
    wa_d = nc.declare_dram_parameter("wa", [NITER, 3, 2, 2, 128, 128], BF16,
                                     isOutput=False)
    b3_d = nc.declare_dram_parameter("b3", [NITER, 2, 128, 1], F32,
                                     isOutput=False)
    ixr_d = nc.declare_dram_parameter("ixrel", [NITER, nR, 512], mybir.dt.int8,
                                      isOutput=False)
    iota_d = nc.declare_dram_parameter("iota", [128, 1], F32, isOutput=False)
    id_d = nc.declare_dram_parameter("ident", [128, 128], BF16, isOutput=False)
    if has_b1:
        b1r_d = nc.declare_dram_parameter("b1row", [NITER, 1, 256], BF16,
                                          isOutput=False)
        ones_d = nc.declare_dram_parameter("ones", [1, 128], BF16,
                                           isOutput=False)
    out_d = nc.declare_dram_parameter("out", [2, 128, T], F32, isOutput=True)

    with tile.TileContext(nc) as tc, ExitStack() as ctx:
        xpool = ctx.enter_context(tc.tile_pool(name="xres", bufs=1))
        stp = ctx.enter_context(tc.tile_pool(name="stp", bufs=LAG + 4))
        xtp = ctx.enter_context(tc.tile_pool(name="xtp", bufs=nT))
        vp = ctx.enter_context(tc.tile_pool(name="vp", bufs=nT))
        vtp = ctx.enter_context(tc.tile_pool(name="vtp", bufs=4))
        relp = ctx.enter_context(tc.tile_pool(name="relp", bufs=6))
        sp = ctx.enter_context(tc.tile_pool(name="sp", bufs=6))
        wts = ctx.enter_context(tc.tile_pool(name="wts", bufs=2))
        cst = ctx.enter_context(tc.tile_pool(name="cst", bufs=1))
        pu_ps = ctx.enter_context(tc.tile_pool(name="pu", bufs=2, space="PSUM"))
        pv_ps = ctx.enter_context(tc.tile_pool(name="pv", bufs=2, space="PSUM"))
        pt_ps = ctx.enter_context(tc.tile_pool(name="pt", bufs=4, space="PSUM"))

        def load_weights(i):
            # on the sync ring: DMA issues on the ACT queue would delay the
            # first st-copies behind their DGE setup
            wpf_sb = wts.tile([128, 2, 512], BF16, tag="wpf")
            nc.sync.dma_start(wpf_sb[:, :, :],
                              wpf_d[i].rearrange("c p n -> p c n"))
            wct_sb = wts.tile([128, 2, 256], BF16, tag="wct")
            nc.sync.dma_start(wct_sb[:, :, :],
                              wct_d[i].rearrange("c p n -> p c n"))
            wa_sb = wts.tile([128, 3, 2, 2, 128], BF16, tag="wa")
            nc.sync.dma_start(wa_sb[:, :, :, :, :],
                              wa_d[i].rearrange("k c o p f -> p k c o f"))
            b3_sb = wts.tile([128, 2], F32, tag="b3")
            nc.sync.dma_start(b3_sb[:, :],
                              b3_d[i].rearrange("o p u -> p (o u)"))
            b1r_sb = None
            if has_b1:
                b1r_sb = wts.tile([1, 256], BF16, tag="b1r")
                nc.sync.dma_start(b1r_sb[:, :], b1r_d[i])
            return wpf_sb, wct_sb, wa_sb, b3_sb, b1r_sb

        # Head staging: the first columns of x land in a small dedicated
        # tile with an unambiguous early dependency, so iteration 0 starts
        # without waiting for the bulk x load. Small pieces so the very
        # first tile arrives within a couple of microseconds.
        HEAD = min(4096, T)
        x_head = cst.tile([128, 2, HEAD], F32)
        for sg in range(HEAD // 512):
            sl = bass.ts(sg, 512)
            nc.sync.dma_start(x_head[:, :, sl],
                              x_d[:, :, sl].rearrange("c p n -> p c n"))
        w0 = load_weights(0)
        iota_sb = cst.tile([128, 1], F32)
        nc.sync.dma_start(iota_sb[:, :], iota_d[:, :])
        ident_sb = cst.tile([128, 128], BF16)
        nc.sync.dma_start(ident_sb[:, :], id_d[:, :])
        if has_b1:
            ones_sb = cst.tile([1, 128], BF16)
            nc.sync.dma_start(ones_sb[:, :], ones_d[:, :])

        # Resident fp32 signal.  Columns [0, HEAD) are served by x_head in
        # iteration 0 and fully written by its residuals, so skip them here.
        # The bulk load runs on the gpsimd+vector rings, keeping the sync
        # queue free for the rel broadcasts that gate the first S builds.
        x_sb = xpool.tile([128, 2, T], F32)
        for sg in range((T - HEAD) // 512):
            sl = bass.ds(HEAD + sg * 512, 512)
            for cb in range(2):
                nc.gpsimd.dma_start(x_sb[:, cb, sl], x_d[cb, :, sl])

        st_tiles = [None] * nR   # rolling u^T stripe tiles
        xt_tiles = [None] * nT
        xt_next = [None] * nT
        v_tiles = [None] * nT
        vt_tiles = [None] * (nR // 2)
        pv_tiles = [None] * (nR // 2)

        def emit_A(tt, head=False, pieces=1):
            src0 = x_head if head else x_sb
            xt = xtp.tile([128, 2, 512], BF16, tag="xt")
            for p in range(pieces):
                w = 512 // pieces
                sl = bass.ds(tt * 512 + p * w, w)
                src = src0[:, :, sl]
                nc.vector.scalar_tensor_tensor(
                    xt[:, :, p * w:(p + 1) * w], src, SLOPE, src,
                    OP.mult, OP.max)
            return xt

        def emit_conv3(tt, wa_sb, b3_sb, last, head=False):
            tsl = bass.ts(tt, 512)
            for ob in range(2):
                py = pt_ps.tile([128, 512], F32, tag="pt")
                j = 0
                for k in range(3):
                    for cb in range(2):
                        nc.tensor.matmul(py[:, :], wa_sb[:, k, cb, ob, :],
                                         v_tiles[tt][:, cb, k:k + 512],
                                         start=(j == 0), stop=(j == 5))
                        j += 1
                # residual:  x = (y + b3) + x  (iter-0 head tiles read the
                # staging copy, write the resident tensor)
                xin = x_head[:, ob, tsl] if head else x_sb[:, ob, tsl]
                nc.vector.scalar_tensor_tensor(
                    x_sb[:, ob, tsl], py[:, :], b3_sb[:, ob:ob + 1],
                    xin, OP.add, OP.add)
            if last:
                for cb in range(2):
                    nc.sync.dma_start(out_d[cb, :, tsl], x_sb[:, cb, tsl])

        for i in range(NITER):
            wpf_sb, wct_sb, wa_sb, b3_sb, b1r_sb = \
                w0 if i == 0 else load_weights(i)

            if i > 0:
                xt_tiles, xt_next = xt_next, [None] * nT
            else:
                # iteration-0 lrelus run two block-groups ahead of use (not
                # all upfront: 16 queued lrelus would delay the S builds that
                # gate the gather matmuls by ~20us)
                xt_tiles[0] = emit_A(0, head=True, pieces=4)
                xt_tiles[1] = emit_A(1, head=True, pieces=2)

            def emit_B(b):
                tt, off = b // 4, (b % 4) * 128
                ps = pu_ps.tile([128, 512], F32, tag="pu")
                nc.tensor.matmul(ps[:, :], xt_tiles[tt][:, 0, off:off + 128],
                                 wpf_sb[:, 0, :], start=True, stop=False)
                nc.tensor.matmul(ps[:, :], xt_tiles[tt][:, 1, off:off + 128],
                                 wpf_sb[:, 1, :], start=False, stop=True)
                st = stp.tile([128, 512], BF16, tag="st")
                st_tiles[b] = st
                nc.scalar.activation(st[:, :], ps[:, :], AF.Copy)

            def emit_G(b):
                tt, off = b // 4, (b % 4) * 128
                rel = relp.tile([128, 512], mybir.dt.int8, tag="rel")
                nc.sync.dma_start(rel[:, :],
                                  ixr_d[i, b].partition_broadcast(128))
                S = sp.tile([128, 512], BF16, tag="S")
                nc.vector.tensor_scalar(S[:, :], rel[:, :], iota_sb[:, 0:1],
                                        None, OP.is_equal)
                pr = b // 2
                half = (b % 2) * 256
                if b % 2 == 0:
                    pv_tile = pv_ps.tile([128, 512], F32, tag="pv")
                    pv_tiles[pr] = pv_tile
                pv = pv_tiles[pr]
                out_sl = pv[:, half:half + 256]
                mms = [
                    (xt_tiles[tt][:, 0, off:off + 128], wct_sb[:, 0, :]),
                    (xt_tiles[tt][:, 1, off:off + 128], wct_sb[:, 1, :]),
                ]
                if has_b1:
                    mms.append((ones_sb[:, :], b1r_sb[:, :]))
                if b > 0:
                    mms.append((S[:, 0:128], st_tiles[b - 1][:, 0:256]))
                mms.append((S[:, 128:256], st_tiles[b][:, 0:256]))
                mms.append((S[:, 256:384], st_tiles[b][:, 256:512]))
                if b < nR - 1:
                    mms.append((S[:, 384:512], st_tiles[b + 1][:, 256:512]))
                for j, (lhsT, rhs) in enumerate(mms):
                    nc.tensor.matmul(out_sl, lhsT, rhs, start=(j == 0),
                                     stop=(j == len(mms) - 1))
                if b % 2 == 1:
                    vt = vtp.tile([128, 512], BF16, tag="vt")
                    vt_tiles[pr] = vt
                    nc.scalar.activation(vt[:, :], pv[:, :], AF.Prelu,
                                         alpha=SLOPE)
                if b % 4 == 3:
                    emit_quad(b // 4)

            def emit_quad(q):
                # transpose blocks 4q..4q+3 back to channel-major v tile
                v = vp.tile([128, 2, 516], BF16, tag="v")
                v_tiles[q] = v
                for cb in range(2):
                    pt = pt_ps.tile([128, 512], BF16, tag="pt")
                    for j4 in range(4):
                        vt = vt_tiles[2 * q + j4 // 2]
                        csl = (j4 % 2) * 256 + cb * 128
                        nc.tensor.transpose(pt[:, j4 * 128:(j4 + 1) * 128],
                                            vt[:, csl:csl + 128],
                                            ident_sb[:, :])
                    nc.scalar.activation(v[:, cb, 1:513], pt[:, :], AF.Copy)
                if q == 0:
                    nc.vector.memset(v[:, :, 0:1], 0.0)
                else:
                    nc.vector.tensor_copy(v[:, :, 0:1],
                                          v_tiles[q - 1][:, :, 512:513])
                    nc.vector.tensor_copy(v_tiles[q - 1][:, :, 513:514],
                                          v[:, :, 1:2])
                if q == nT - 1:
                    nc.vector.memset(v[:, :, 513:514], 0.0)
                if q >= 1:
                    emit_conv3(q - 1, wa_sb, b3_sb, i == NITER - 1,
                               head=(i == 0 and (q - 1) * 512 < HEAD))
                    if i + 1 < NITER:
                        xt_next[q - 1] = emit_A(q - 1)

            for b in range(nR + LAG):
                if b < nR:
                    if b % 4 == 0:
                        t2 = b // 4 + 2
                        if t2 < nT and xt_tiles[t2] is None:
                            xt_tiles[t2] = emit_A(t2, head=(t2 * 512 < HEAD))
                    emit_B(b)
                if b >= LAG:
                    emit_G(b - LAG)
            emit_conv3(nT - 1, wa_sb, b3_sb, i == NITER - 1,
                       head=(i == 0 and (nT - 1) * 512 < HEAD))
            if i + 1 < NITER:
                xt_next[nT - 1] = emit_A(nT - 1)

    nc.compile()
    return nc


def _to_bf16(a):
    return np.asarray(a, dtype=np.float32).astype(ml_dtypes.bfloat16)


def prep_in_maps(x, d, WC, bC, WP, bP, WF, bF, WA, bA, T=T_FULL):
    """Build the 8 per-core input maps from the full-problem arrays.
    Returns (in_maps, has_b1)."""
    x = np.asarray(x, dtype=np.float32)
    d = np.asarray(d, dtype=np.float32)
    WC, WP, WF, WA = (np.asarray(w, dtype=np.float32) for w in (WC, WP, WF, WA))
    bC, bP, bF, bA = (np.asarray(b, dtype=np.float32) for b in (bC, bP, bF, bA))
    nb = x.shape[0]
    nR = T // 128

    wpf = np.empty((NITER, 2, 128, 512), np.float32)
    wct = np.empty((NITER, 2, 128, 256), np.float32)
    wa = np.empty((NITER, 3, 2, 2, 128, 128), np.float32)
    for i in range(NITER):
        wpfT = np.concatenate([WP[i].T, WF[i].T], axis=1)  # [c', 512]
        wpf[i] = wpfT.reshape(2, 128, 512)
        wct[i] = WC[i].T.reshape(2, 128, 256)              # [cc, p, o]
        for k in range(3):
            waT = WA[i, :, :, k].T                         # [c', o]
            wa[i, k] = waT.reshape(2, 128, 2, 128).transpose(0, 2, 1, 3)
    b1 = (bC + bP + bF).astype(np.float32)                  # [NITER, 256]
    has_b1 = bool(np.any(b1 != 0))
    b3 = bA.reshape(NITER, 2, 128, 1).astype(np.float32)

    wpf, wct, wa = _to_bf16(wpf), _to_bf16(wct), _to_bf16(wa)
    iota = np.arange(128, dtype=np.float32).reshape(128, 1)
    ident = _to_bf16(np.eye(128, dtype=np.float32))

    tf = np.arange(T, dtype=np.float32)
    in_maps = []
    for b in range(nb):
        dv = d[b, 0].astype(np.float32)
        ixr = np.full((NITER, nR, 512), INVALID, np.int16)
        for i, dil in enumerate(DILATIONS):
            dd = dv * np.float32(dil)
            rp = np.round(tf - dd).astype(np.int64)
            rf = np.round(tf + dd).astype(np.int64)
            rp = np.where(rp >= 0, rp, np.int64(-(1 << 32)))
            rf = np.where(rf < T, rf, np.int64(1 << 32))
            blk = np.arange(nR).repeat(128) * 128           # (T,)
            for c, base in enumerate((rp - blk + 128, rp - blk,
                                      rf - blk, rf - blk - 128)):
                v = np.where((base >= 0) & (base < 128), base,
                             np.int64(INVALID)).astype(np.int16)
                ixr[i, :, c * 128:(c + 1) * 128] = v.reshape(nR, 128)
        m = {
            "x": x[b].reshape(2, 128, T).copy(),
            "wpf": wpf, "wct": wct, "wa": wa, "b3": b3,
            "ixrel": np.clip(ixr, -128, 127).astype(np.int8),
            "iota": iota, "ident": ident,
        }
        if has_b1:
            m["b1row"] = _to_bf16(b1.reshape(NITER, 1, 256))
            m["ones"] = _to_bf16(np.ones((1, 128), np.float32))
        in_maps.append(m)
    return in_maps, has_b1


_nc_cache = {}


def kernel(**inputs) -> np.ndarray:
    T = inputs["x"].shape[2]
    in_maps, has_b1 = prep_in_maps(**inputs, T=T)
    key = (T, has_b1)
    if key not in _nc_cache:
        _nc_cache[key] = build_nc(T, has_b1=has_b1)
    nc = _nc_cache[key]
    res = run_bass_kernel_spmd(nc, in_maps, core_ids=list(range(8)))
    out = np.stack([np.asarray(res.results[i]["out"], dtype=np.float32)
                    .reshape(C, T) for i in range(8)])
    return out



# revision 29
# speedup vs baseline: 3.6248x; 1.0017x over previous
"""Trainium2 Bass kernel for nn_AdaptiveResBlock (8-core data-parallel).

Reference computation (per batch element b, C=256 channels, T=8192 time):
  for i, dil in enumerate((1, 2, 4)):
      xt = lrelu(x)
      xP, xF = time-gather of xt at round(t -/+ d*dil), zero out-of-range
      xt = WC@xt + WP@xP + WF@xF + biases        (1x1 convs over channels)
      xt = lrelu(xt)
      xt = conv3(xt, WA) + bias
      x = xt + x

Structure used:
  * The time-gather commutes with the 1x1 convs (indices are per-time,
    shared across channels):  WP @ gather(xt) == gather(WP @ xt).
  * Gather offsets are bounded by max(d)*dil <= 64 < 128, so for output
    token block b (128 tokens) the P-gather sources live in token blocks
    {b-1, b} and the F-gather sources in {b, b+1}.
  * The gather itself runs on the TensorEngine as one-hot matmuls in
    token-major space: S[j, t] = (idx(t) == j) built on DVE via is_equal
    against an iota column (relative indices DMA-broadcast from DRAM);
    out-of-range indices get no one-hot bit => free zero masking.
  * Per token block the PSUM accumulation fuses: uC^T (xt-stationary
    matmuls) + one-hot gathers of uP^T / uF^T (+ optional bias rank-1
    matmul).  ACT applies leaky-relu (Prelu, exact on HW) straight from
    PSUM; PE transposes v^T back to channel-major for the 3-tap conv.
  * The u^T token stripe is a rolling window of per-block SBUF tiles
    (B-stage runs a few blocks ahead of the gather stage).
  * lrelu(x) for the next iteration is emitted right behind each tile's
    residual update so the next iteration's matmuls start immediately.

Sharded data-parallel over B=8 across the 8 NeuronCores; (C,C) weights
replicated; per-core relative-gather-index tensors precomputed from d.
"""

import numpy as np
import ml_dtypes
from contextlib import ExitStack

import concourse.bass as bass
import concourse.tile as tile
from concourse import mybir, bacc
from concourse.bass_utils import run_bass_kernel_spmd

F32 = mybir.dt.float32
BF16 = mybir.dt.bfloat16
I16 = mybir.dt.int16
AF = mybir.ActivationFunctionType
OP = mybir.AluOpType

B, C, T_FULL = 8, 256, 8192
DILATIONS = (1, 2, 4)
NITER = len(DILATIONS)
SLOPE = 0.1
INVALID = -512
LAG = 3  # B-stage leads the gather stage by this many token blocks


def build_nc(T=T_FULL, num_devices=8, has_b1=False):
    nT = T // 512            # 512-wide time tiles
    nR = T // 128            # 128-wide token blocks

    nc = bacc.Bacc("TRN2", target_bir_lowering=False, debug=False,
                   num_devices=num_devices)
    x_d = nc.declare_dram_parameter("x", [2, 128, T], F32, isOutput=False)
    wpf_d = nc.declare_dram_parameter("wpf", [NITER, 2, 128, 512], BF16,
                                      isOutput=False)
    wct_d = nc.declare_dram_parameter("wct", [NITER, 2, 128, 256], BF16,
                                      isOutput=False)
    wa_d = nc.declare_dram_parameter("wa", [NITER, 3, 2, 2, 128, 128], BF16,
                                     isOutput=False)
    b3_d = nc.declare_dram_parameter("b3", [NITER, 2, 128, 1], F32,
                                     isOutput=False)
    ixr_d = nc.declare_dram_parameter("ixrel", [NITER, nR, 512], mybir.dt.int8,
                                      isOutput=False)
    iota_d = nc.declare_dram_parameter("iota", [128, 1], F32, isOutput=False)
    id_d = nc.declare_dram_parameter("ident", [128, 128], BF16, isOutput=False)
    if has_b1:
        b1r_d = nc.declare_dram_parameter("b1row", [NITER, 1, 256], BF16,
                                          isOutput=False)
        ones_d = nc.declare_dram_parameter("ones", [1, 128], BF16,
                                           isOutput=False)
    out_d = nc.declare_dram_parameter("out", [2, 128, T], F32, isOutput=True)

    with tile.TileContext(nc) as tc, ExitStack() as ctx:
        xpool = ctx.enter_context(tc.tile_pool(name="xres", bufs=1))
        stp = ctx.enter_context(tc.tile_pool(name="stp", bufs=LAG + 4))
        xtp = ctx.enter_context(tc.tile_pool(name="xtp", bufs=nT))
        vp = ctx.enter_context(tc.tile_pool(name="vp", bufs=nT))
        vtp = ctx.enter_context(tc.tile_pool(name="vtp", bufs=4))
        relp = ctx.enter_context(tc.tile_pool(name="relp", bufs=6))
        sp = ctx.enter_context(tc.tile_pool(name="sp", bufs=6))
        wts = ctx.enter_context(tc.tile_pool(name="wts", bufs=2))
        cst = ctx.enter_context(tc.tile_pool(name="cst", bufs=1))
        pu_ps = ctx.enter_context(tc.tile_pool(name="pu", bufs=2, space="PSUM"))
        pv_ps = ctx.enter_context(tc.tile_pool(name="pv", bufs=2, space="PSUM"))
        pt_ps = ctx.enter_context(tc.tile_pool(name="pt", bufs=4, space="PSUM"))

        def load_weights(i):
            # on the sync ring: DMA issues on the ACT queue would delay the
            # first st-copies behind their DGE setup
            wpf_sb = wts.tile([128, 2, 512], BF16, tag="wpf")
            nc.sync.dma_start(wpf_sb[:, :, :],
                              wpf_d[i].rearrange("c p n -> p c n"))
            wct_sb = wts.tile([128, 2, 256], BF16, tag="wct")
            nc.sync.dma_start(wct_sb[:, :, :],
                              wct_d[i].rearrange("c p n -> p c n"))
            wa_sb = wts.tile([128, 3, 2, 2, 128], BF16, tag="wa")
            nc.sync.dma_start(wa_sb[:, :, :, :, :],
                              wa_d[i].rearrange("k c o p f -> p k c o f"))
            b3_sb = wts.tile([128, 2], F32, tag="b3")
            nc.sync.dma_start(b3_sb[:, :],
                              b3_d[i].rearrange("o p u -> p (o u)"))
            b1r_sb = None
            if has_b1:
                b1r_sb = wts.tile([1, 256], BF16, tag="b1r")
                nc.sync.dma_start(b1r_sb[:, :], b1r_d[i])
            return wpf_sb, wct_sb, wa_sb, b3_sb, b1r_sb

        # Head staging: the first columns of x land in a small dedicated
        # tile with an unambiguous early dependency, so iteration 0 starts
        # without waiting for the bulk x load. Small pieces so the very
        # first tile arrives within a couple of microseconds.
        HEAD = min(2048, T)
        x_head = cst.tile([128, 2, HEAD], F32)
        for sg in range(HEAD // 512):
            sl = bass.ts(sg, 512)
            nc.sync.dma_start(x_head[:, :, sl],
                              x_d[:, :, sl].rearrange("c p n -> p c n"))
        w0 = load_weights(0)
        iota_sb = cst.tile([128, 1], F32)
        nc.sync.dma_start(iota_sb[:, :], iota_d[:, :])
        ident_sb = cst.tile([128, 128], BF16)
        nc.sync.dma_start(ident_sb[:, :], id_d[:, :])
        if has_b1:
            ones_sb = cst.tile([1, 128], BF16)
            nc.sync.dma_start(ones_sb[:, :], ones_d[:, :])

        # Resident fp32 signal.  Columns [0, HEAD) are served by x_head in
        # iteration 0 and fully written by its residuals, so skip them here.
        # The bulk load runs on the gpsimd+vector rings, keeping the sync
        # queue free for the rel broadcasts that gate the first S builds.
        x_sb = xpool.tile([128, 2, T], F32)
        for sg in range((T - HEAD) // 512):
            sl = bass.ds(HEAD + sg * 512, 512)
            for cb in range(2):
                nc.gpsimd.dma_start(x_sb[:, cb, sl], x_d[cb, :, sl])

        st_tiles = [None] * nR   # rolling u^T stripe tiles
        xt_tiles = [None] * nT
        xt_next = [None] * nT
        v_tiles = [None] * nT
        vt_tiles = [None] * (nR // 2)
        pv_tiles = [None] * (nR // 2)

        def emit_A(tt, head=False, pieces=1):
            src0 = x_head if head else x_sb
            xt = xtp.tile([128, 2, 512], BF16, tag="xt")
            for p in range(pieces):
                w = 512 // pieces
                sl = bass.ds(tt * 512 + p * w, w)
                src = src0[:, :, sl]
                nc.vector.scalar_tensor_tensor(
                    xt[:, :, p * w:(p + 1) * w], src, SLOPE, src,
                    OP.mult, OP.max)
            return xt

        def emit_conv3(tt, wa_sb, b3_sb, last, head=False):
            tsl = bass.ts(tt, 512)
            for ob in range(2):
                py = pt_ps.tile([128, 512], F32, tag="pt")
                j = 0
                for k in range(3):
                    for cb in range(2):
                        nc.tensor.matmul(py[:, :], wa_sb[:, k, cb, ob, :],
                                         v_tiles[tt][:, cb, k:k + 512],
                                         start=(j == 0), stop=(j == 5))
                        j += 1
                # residual:  x = (y + b3) + x  (iter-0 head tiles read the
                # staging copy, write the resident tensor)
                xin = x_head[:, ob, tsl] if head else x_sb[:, ob, tsl]
                nc.vector.scalar_tensor_tensor(
                    x_sb[:, ob, tsl], py[:, :], b3_sb[:, ob:ob + 1],
                    xin, OP.add, OP.add)
            if last:
                for cb in range(2):
                    nc.sync.dma_start(out_d[cb, :, tsl], x_sb[:, cb, tsl])

        for i in range(NITER):
            wpf_sb, wct_sb, wa_sb, b3_sb, b1r_sb = \
                w0 if i == 0 else load_weights(i)

            if i > 0:
                xt_tiles, xt_next = xt_next, [None] * nT
            else:
                # iteration-0 lrelus run two block-groups ahead of use (not
                # all upfront: 16 queued lrelus would delay the S builds that
                # gate the gather matmuls by ~20us)
                xt_tiles[0] = emit_A(0, head=True, pieces=4)
                xt_tiles[1] = emit_A(1, head=True, pieces=2)

            def emit_B(b):
                tt, off = b // 4, (b % 4) * 128
                ps = pu_ps.tile([128, 512], F32, tag="pu")
                nc.tensor.matmul(ps[:, :], xt_tiles[tt][:, 0, off:off + 128],
                                 wpf_sb[:, 0, :], start=True, stop=False)
                nc.tensor.matmul(ps[:, :], xt_tiles[tt][:, 1, off:off + 128],
                                 wpf_sb[:, 1, :], start=False, stop=True)
                st = stp.tile([128, 512], BF16, tag="st")
                st_tiles[b] = st
                nc.scalar.activation(st[:, :], ps[:, :], AF.Copy)

            def emit_G(b):
                tt, off = b // 4, (b % 4) * 128
                rel = relp.tile([128, 512], mybir.dt.int8, tag="rel")
                nc.sync.dma_start(rel[:, :],
                                  ixr_d[i, b].partition_broadcast(128))
                S = sp.tile([128, 512], BF16, tag="S")
                nc.vector.tensor_scalar(S[:, :], rel[:, :], iota_sb[:, 0:1],
                                        None, OP.is_equal)
                pr = b // 2
                half = (b % 2) * 256
                if b % 2 == 0:
                    pv_tile = pv_ps.tile([128, 512], F32, tag="pv")
                    pv_tiles[pr] = pv_tile
                pv = pv_tiles[pr]
                out_sl = pv[:, half:half + 256]
                mms = [
                    (xt_tiles[tt][:, 0, off:off + 128], wct_sb[:, 0, :]),
                    (xt_tiles[tt][:, 1, off:off + 128], wct_sb[:, 1, :]),
                ]
                if has_b1:
                    mms.append((ones_sb[:, :], b1r_sb[:, :]))
                if b > 0:
                    mms.append((S[:, 0:128], st_tiles[b - 1][:, 0:256]))
                mms.append((S[:, 128:256], st_tiles[b][:, 0:256]))
                mms.append((S[:, 256:384], st_tiles[b][:, 256:512]))
                if b < nR - 1:
                    mms.append((S[:, 384:512], st_tiles[b + 1][:, 256:512]))
                for j, (lhsT, rhs) in enumerate(mms):
                    nc.tensor.matmul(out_sl, lhsT, rhs, start=(j == 0),
                                     stop=(j == len(mms) - 1))
                if b % 2 == 1:
                    vt = vtp.tile([128, 512], BF16, tag="vt")
                    vt_tiles[pr] = vt
                    nc.scalar.activation(vt[:, :], pv[:, :], AF.Prelu,
                                         alpha=SLOPE)
                if b % 4 == 3:
                    emit_quad(b // 4)

            def emit_quad(q):
                # transpose blocks 4q..4q+3 back to channel-major v tile
                v = vp.tile([128, 2, 516], BF16, tag="v")
                v_tiles[q] = v
                for cb in range(2):
                    pt = pt_ps.tile([128, 512], BF16, tag="pt")
                    for j4 in range(4):
                        vt = vt_tiles[2 * q + j4 // 2]
                        csl = (j4 % 2) * 256 + cb * 128
                        nc.tensor.transpose(pt[:, j4 * 128:(j4 + 1) * 128],
                                            vt[:, csl:csl + 128],
                                            ident_sb[:, :])
                    nc.scalar.activation(v[:, cb, 1:513], pt[:, :], AF.Copy)
                if q == 0:
                    nc.vector.memset(v[:, :, 0:1], 0.0)
                else:
                    nc.vector.tensor_copy(v[:, :, 0:1],
                                          v_tiles[q - 1][:, :, 512:513])
                    nc.vector.tensor_copy(v_tiles[q - 1][:, :, 513:514],
                                          v[:, :, 1:2])
                if q == nT - 1:
                    nc.vector.memset(v[:, :, 513:514], 0.0)
                if q >= 1:
                    emit_conv3(q - 1, wa_sb, b3_sb, i == NITER - 1,
                               head=(i == 0 and (q - 1) * 512 < HEAD))
                    if i + 1 < NITER:
                        xt_next[q - 1] = emit_A(q - 1)

            for b in range(nR + LAG):
                if b < nR:
                    if b % 4 == 0:
                        t2 = b // 4 + 2
                        if t2 < nT and xt_tiles[t2] is None:
                            xt_tiles[t2] = emit_A(t2, head=(t2 * 512 < HEAD))
                    emit_B(b)
                if b >= LAG:
                    emit_G(b - LAG)
            emit_conv3(nT - 1, wa_sb, b3_sb, i == NITER - 1,
                       head=(i == 0 and (nT - 1) * 512 < HEAD))
            if i + 1 < NITER:
                xt_next[nT - 1] = emit_A(nT - 1)

    nc.compile()
    return nc


def _to_bf16(a):
    return np.asarray(a, dtype=np.float32).astype(ml_dtypes.bfloat16)


def prep_in_maps(x, d, WC, bC, WP, bP, WF, bF, WA, bA, T=T_FULL):
    """Build the 8 per-core input maps from the full-problem arrays.
    Returns (in_maps, has_b1)."""
    x = np.asarray(x, dtype=np.float32)
    d = np.asarray(d, dtype=np.float32)
    WC, WP, WF, WA = (np.asarray(w, dtype=np.float32) for w in (WC, WP, WF, WA))
    bC, bP, bF, bA = (np.asarray(b, dtype=np.float32) for b in (bC, bP, bF, bA))
    nb = x.shape[0]
    nR = T // 128

    wpf = np.empty((NITER, 2, 128, 512), np.float32)
    wct = np.empty((NITER, 2, 128, 256), np.float32)
    wa = np.empty((NITER, 3, 2, 2, 128, 128), np.float32)
    for i in range(NITER):
        wpfT = np.concatenate([WP[i].T, WF[i].T], axis=1)  # [c', 512]
        wpf[i] = wpfT.reshape(2, 128, 512)
        wct[i] = WC[i].T.reshape(2, 128, 256)              # [cc, p, o]
        for k in range(3):
            waT = WA[i, :, :, k].T                         # [c', o]
            wa[i, k] = waT.reshape(2, 128, 2, 128).transpose(0, 2, 1, 3)
    b1 = (bC + bP + bF).astype(np.float32)                  # [NITER, 256]
    has_b1 = bool(np.any(b1 != 0))
    b3 = bA.reshape(NITER, 2, 128, 1).astype(np.float32)

    wpf, wct, wa = _to_bf16(wpf), _to_bf16(wct), _to_bf16(wa)
    iota = np.arange(128, dtype=np.float32).reshape(128, 1)
    ident = _to_bf16(np.eye(128, dtype=np.float32))

    tf = np.arange(T, dtype=np.float32)
    in_maps = []
    for b in range(nb):
        dv = d[b, 0].astype(np.float32)
        ixr = np.full((NITER, nR, 512), INVALID, np.int16)
        for i, dil in enumerate(DILATIONS):
            dd = dv * np.float32(dil)
            rp = np.round(tf - dd).astype(np.int64)
            rf = np.round(tf + dd).astype(np.int64)
            rp = np.where(rp >= 0, rp, np.int64(-(1 << 32)))
            rf = np.where(rf < T, rf, np.int64(1 << 32))
            blk = np.arange(nR).repeat(128) * 128           # (T,)
            for c, base in enumerate((rp - blk + 128, rp - blk,
                                      rf - blk, rf - blk - 128)):
                v = np.where((base >= 0) & (base < 128), base,
                             np.int64(INVALID)).astype(np.int16)
                ixr[i, :, c * 128:(c + 1) * 128] = v.reshape(nR, 128)
        m = {
            "x": x[b].reshape(2, 128, T).copy(),
            "wpf": wpf, "wct": wct, "wa": wa, "b3": b3,
            "ixrel": np.clip(ixr, -128, 127).astype(np.int8),
            "iota": iota, "ident": ident,
        }
        if has_b1:
            m["b1row"] = _to_bf16(b1.reshape(NITER, 1, 256))
            m["ones"] = _to_bf16(np.ones((1, 128), np.float32))
        in_maps.append(m)
    return in_maps, has_b1


_nc_cache = {}


def kernel(**inputs) -> np.ndarray:
    T = inputs["x"].shape[2]
    in_maps, has_b1 = prep_in_maps(**inputs, T=T)
    key = (T, has_b1)
    if key not in _nc_cache:
        _nc_cache[key] = build_nc(T, has_b1=has_b1)
    nc = _nc_cache[key]
    res = run_bass_kernel_spmd(nc, in_maps, core_ids=list(range(8)))
    out = np.stack([np.asarray(res.results[i]["out"], dtype=np.float32)
                    .reshape(C, T) for i in range(8)])
    return out



# revision 30
# speedup vs baseline: 3.6582x; 1.0092x over previous
"""Trainium2 Bass kernel for nn_AdaptiveResBlock (8-core data-parallel).

Reference computation (per batch element b, C=256 channels, T=8192 time):
  for i, dil in enumerate((1, 2, 4)):
      xt = lrelu(x)
      xP, xF = time-gather of xt at round(t -/+ d*dil), zero out-of-range
      xt = WC@xt + WP@xP + WF@xF + biases        (1x1 convs over channels)
      xt = lrelu(xt)
      xt = conv3(xt, WA) + bias
      x = xt + x

Structure used:
  * The time-gather commutes with the 1x1 convs (indices are per-time,
    shared across channels):  WP @ gather(xt) == gather(WP @ xt).
  * Gather offsets are bounded by max(d)*dil <= 64 < 128, so for output
    token block b (128 tokens) the P-gather sources live in token blocks
    {b-1, b} and the F-gather sources in {b, b+1}.
  * The gather itself runs on the TensorEngine as one-hot matmuls in
    token-major space: S[j, t] = (idx(t) == j) built on DVE via is_equal
    against an iota column (relative indices DMA-broadcast from DRAM);
    out-of-range indices get no one-hot bit => free zero masking.
  * Per token block the PSUM accumulation fuses: uC^T (xt-stationary
    matmuls) + one-hot gathers of uP^T / uF^T (+ optional bias rank-1
    matmul).  ACT applies leaky-relu (Prelu, exact on HW) straight from
    PSUM; PE transposes v^T back to channel-major for the 3-tap conv.
  * The u^T token stripe is a rolling window of per-block SBUF tiles
    (B-stage runs a few blocks ahead of the gather stage).
  * lrelu(x) for the next iteration is emitted right behind each tile's
    residual update so the next iteration's matmuls start immediately.

Sharded data-parallel over B=8 across the 8 NeuronCores; (C,C) weights
replicated; per-core relative-gather-index tensors precomputed from d.
"""

import numpy as np
import ml_dtypes
from contextlib import ExitStack

import concourse.bass as bass
import concourse.tile as tile
from concourse import mybir, bacc
from concourse.bass_utils import run_bass_kernel_spmd

F32 = mybir.dt.float32
BF16 = mybir.dt.bfloat16
I16 = mybir.dt.int16
AF = mybir.ActivationFunctionType
OP = mybir.AluOpType

B, C, T_FULL = 8, 256, 8192
DILATIONS = (1, 2, 4)
NITER = len(DILATIONS)
SLOPE = 0.1
INVALID = -512
LAG = 3  # B-stage leads the gather stage by this many token blocks


def build_nc(T=T_FULL, num_devices=8, has_b1=False):
    nT = T // 512            # 512-wide time tiles
    nR = T // 128            # 128-wide token blocks

    nc = bacc.Bacc("TRN2", target_bir_lowering=False, debug=False,
                   num_devices=num_devices)
    x_d = nc.declare_dram_parameter("x", [2, 128, T], F32, isOutput=False)
    xh_d = nc.declare_dram_parameter("xhead", [128, 2, min(2048, T)], F32,
                                     isOutput=False)
    wpf_d = nc.declare_dram_parameter("wpf", [NITER, 128, 2, 512], BF16,
                                      isOutput=False)
    wct_d = nc.declare_dram_parameter("wct", [NITER, 128, 2, 256], BF16,
                                      isOutput=False)
    wa_d = nc.declare_dram_parameter("wa", [NITER, 128, 3, 2, 2, 128], BF16,
                                     isOutput=False)
    b3_d = nc.declare_dram_parameter("b3", [NITER, 128, 2], F32,
                                     isOutput=False)
    ixr_d = nc.declare_dram_parameter("ixrel", [NITER, nR, 512], mybir.dt.int8,
                                      isOutput=False)
    iota_d = nc.declare_dram_parameter("iota", [128, 1], F32, isOutput=False)
    id_d = nc.declare_dram_parameter("ident", [128, 128], BF16, isOutput=False)
    if has_b1:
        b1r_d = nc.declare_dram_parameter("b1row", [NITER, 1, 256], BF16,
                                          isOutput=False)
        ones_d = nc.declare_dram_parameter("ones", [1, 128], BF16,
                                           isOutput=False)
    out_d = nc.declare_dram_parameter("out", [2, 128, T], F32, isOutput=True)

    with tile.TileContext(nc) as tc, ExitStack() as ctx:
        xpool = ctx.enter_context(tc.tile_pool(name="xres", bufs=1))
        stp = ctx.enter_context(tc.tile_pool(name="stp", bufs=LAG + 4))
        xtp = ctx.enter_context(tc.tile_pool(name="xtp", bufs=nT))
        vp = ctx.enter_context(tc.tile_pool(name="vp", bufs=nT))
        vtp = ctx.enter_context(tc.tile_pool(name="vtp", bufs=4))
        relp = ctx.enter_context(tc.tile_pool(name="relp", bufs=6))
        sp = ctx.enter_context(tc.tile_pool(name="sp", bufs=6))
        wts = ctx.enter_context(tc.tile_pool(name="wts", bufs=2))
        cst = ctx.enter_context(tc.tile_pool(name="cst", bufs=1))
        pu_ps = ctx.enter_context(tc.tile_pool(name="pu", bufs=2, space="PSUM"))
        pv_ps = ctx.enter_context(tc.tile_pool(name="pv", bufs=2, space="PSUM"))
        pt_ps = ctx.enter_context(tc.tile_pool(name="pt", bufs=4, space="PSUM"))

        def load_weights(i):
            # on the sync ring: DMA issues on the ACT queue would delay the
            # first st-copies behind their DGE setup
            wpf_sb = wts.tile([128, 2, 512], BF16, tag="wpf")
            nc.sync.dma_start(wpf_sb[:, :, :], wpf_d[i])
            wct_sb = wts.tile([128, 2, 256], BF16, tag="wct")
            nc.sync.dma_start(wct_sb[:, :, :], wct_d[i])
            wa_sb = wts.tile([128, 3, 2, 2, 128], BF16, tag="wa")
            nc.sync.dma_start(wa_sb[:, :, :, :, :], wa_d[i])
            b3_sb = wts.tile([128, 2], F32, tag="b3")
            nc.sync.dma_start(b3_sb[:, :], b3_d[i])
            b1r_sb = None
            if has_b1:
                b1r_sb = wts.tile([1, 256], BF16, tag="b1r")
                nc.sync.dma_start(b1r_sb[:, :], b1r_d[i])
            return wpf_sb, wct_sb, wa_sb, b3_sb, b1r_sb

        # Head staging: the first columns of x land in a small dedicated
        # tile with an unambiguous early dependency, so iteration 0 starts
        # without waiting for the bulk x load. Small pieces so the very
        # first tile arrives within a couple of microseconds.
        HEAD = min(2048, T)
        x_head = cst.tile([128, 2, HEAD], F32)
        for sg in range(HEAD // 512):
            sl = bass.ts(sg, 512)
            nc.sync.dma_start(x_head[:, :, sl], xh_d[:, :, sl])
        w0 = load_weights(0)
        iota_sb = cst.tile([128, 1], F32)
        nc.sync.dma_start(iota_sb[:, :], iota_d[:, :])
        ident_sb = cst.tile([128, 128], BF16)
        nc.sync.dma_start(ident_sb[:, :], id_d[:, :])
        if has_b1:
            ones_sb = cst.tile([1, 128], BF16)
            nc.sync.dma_start(ones_sb[:, :], ones_d[:, :])

        # Resident fp32 signal.  Columns [0, HEAD) are served by x_head in
        # iteration 0 and fully written by its residuals, so skip them here.
        # The bulk load runs on the gpsimd+vector rings, keeping the sync
        # queue free for the rel broadcasts that gate the first S builds.
        x_sb = xpool.tile([128, 2, T], F32)
        for sg in range((T - HEAD) // 512):
            sl = bass.ds(HEAD + sg * 512, 512)
            for cb in range(2):
                nc.gpsimd.dma_start(x_sb[:, cb, sl], x_d[cb, :, sl])

        st_tiles = [None] * nR   # rolling u^T stripe tiles
        xt_tiles = [None] * nT
        xt_next = [None] * nT
        v_tiles = [None] * nT
        vt_tiles = [None] * (nR // 2)
        pv_tiles = [None] * (nR // 2)

        def emit_A(tt, head=False, pieces=1):
            src0 = x_head if head else x_sb
            xt = xtp.tile([128, 2, 512], BF16, tag="xt")
            for p in range(pieces):
                w = 512 // pieces
                sl = bass.ds(tt * 512 + p * w, w)
                src = src0[:, :, sl]
                nc.vector.scalar_tensor_tensor(
                    xt[:, :, p * w:(p + 1) * w], src, SLOPE, src,
                    OP.mult, OP.max)
            return xt

        def emit_conv3(tt, wa_sb, b3_sb, last, head=False):
            tsl = bass.ts(tt, 512)
            for ob in range(2):
                py = pt_ps.tile([128, 512], F32, tag="pt")
                j = 0
                for k in range(3):
                    for cb in range(2):
                        nc.tensor.matmul(py[:, :], wa_sb[:, k, cb, ob, :],
                                         v_tiles[tt][:, cb, k:k + 512],
                                         start=(j == 0), stop=(j == 5))
                        j += 1
                # residual:  x = (y + b3) + x  (iter-0 head tiles read the
                # staging copy, write the resident tensor)
                xin = x_head[:, ob, tsl] if head else x_sb[:, ob, tsl]
                nc.vector.scalar_tensor_tensor(
                    x_sb[:, ob, tsl], py[:, :], b3_sb[:, ob:ob + 1],
                    xin, OP.add, OP.add)
            if last:
                for cb in range(2):
                    nc.sync.dma_start(out_d[cb, :, tsl], x_sb[:, cb, tsl])

        for i in range(NITER):
            wpf_sb, wct_sb, wa_sb, b3_sb, b1r_sb = \
                w0 if i == 0 else load_weights(i)

            if i > 0:
                xt_tiles, xt_next = xt_next, [None] * nT
            else:
                # iteration-0 lrelus run two block-groups ahead of use (not
                # all upfront: 16 queued lrelus would delay the S builds that
                # gate the gather matmuls by ~20us)
                xt_tiles[0] = emit_A(0, head=True, pieces=4)
                xt_tiles[1] = emit_A(1, head=True, pieces=2)

            def emit_B(b):
                tt, off = b // 4, (b % 4) * 128
                ps = pu_ps.tile([128, 512], F32, tag="pu")
                nc.tensor.matmul(ps[:, :], xt_tiles[tt][:, 0, off:off + 128],
                                 wpf_sb[:, 0, :], start=True, stop=False)
                nc.tensor.matmul(ps[:, :], xt_tiles[tt][:, 1, off:off + 128],
                                 wpf_sb[:, 1, :], start=False, stop=True)
                st = stp.tile([128, 512], BF16, tag="st")
                st_tiles[b] = st
                nc.scalar.activation(st[:, :], ps[:, :], AF.Copy)

            def emit_G(b):
                tt, off = b // 4, (b % 4) * 128
                rel = relp.tile([128, 512], mybir.dt.int8, tag="rel")
                nc.sync.dma_start(rel[:, :],
                                  ixr_d[i, b].partition_broadcast(128))
                S = sp.tile([128, 512], BF16, tag="S")
                nc.vector.tensor_scalar(S[:, :], rel[:, :], iota_sb[:, 0:1],
                                        None, OP.is_equal)
                pr = b // 2
                half = (b % 2) * 256
                if b % 2 == 0:
                    pv_tile = pv_ps.tile([128, 512], F32, tag="pv")
                    pv_tiles[pr] = pv_tile
                pv = pv_tiles[pr]
                out_sl = pv[:, half:half + 256]
                mms = [
                    (xt_tiles[tt][:, 0, off:off + 128], wct_sb[:, 0, :]),
                    (xt_tiles[tt][:, 1, off:off + 128], wct_sb[:, 1, :]),
                ]
                if has_b1:
                    mms.append((ones_sb[:, :], b1r_sb[:, :]))
                if b > 0:
                    mms.append((S[:, 0:128], st_tiles[b - 1][:, 0:256]))
                mms.append((S[:, 128:256], st_tiles[b][:, 0:256]))
                mms.append((S[:, 256:384], st_tiles[b][:, 256:512]))
                if b < nR - 1:
                    mms.append((S[:, 384:512], st_tiles[b + 1][:, 256:512]))
                for j, (lhsT, rhs) in enumerate(mms):
                    nc.tensor.matmul(out_sl, lhsT, rhs, start=(j == 0),
                                     stop=(j == len(mms) - 1))
                if b % 2 == 1:
                    vt = vtp.tile([128, 512], BF16, tag="vt")
                    vt_tiles[pr] = vt
                    nc.scalar.activation(vt[:, :], pv[:, :], AF.Prelu,
                                         alpha=SLOPE)
                if b % 4 == 3:
                    emit_quad(b // 4)

            def emit_quad(q):
                # transpose blocks 4q..4q+3 back to channel-major v tile
                v = vp.tile([128, 2, 516], BF16, tag="v")
                v_tiles[q] = v
                for cb in range(2):
                    pt = pt_ps.tile([128, 512], BF16, tag="pt")
                    for j4 in range(4):
                        vt = vt_tiles[2 * q + j4 // 2]
                        csl = (j4 % 2) * 256 + cb * 128
                        nc.tensor.transpose(pt[:, j4 * 128:(j4 + 1) * 128],
                                            vt[:, csl:csl + 128],
                                            ident_sb[:, :])
                    nc.scalar.activation(v[:, cb, 1:513], pt[:, :], AF.Copy)
                if q == 0:
                    nc.vector.memset(v[:, :, 0:1], 0.0)
                else:
                    nc.vector.tensor_copy(v[:, :, 0:1],
                                          v_tiles[q - 1][:, :, 512:513])
                    nc.vector.tensor_copy(v_tiles[q - 1][:, :, 513:514],
                                          v[:, :, 1:2])
                if q == nT - 1:
                    nc.vector.memset(v[:, :, 513:514], 0.0)
                if q >= 1:
                    emit_conv3(q - 1, wa_sb, b3_sb, i == NITER - 1,
                               head=(i == 0 and (q - 1) * 512 < HEAD))
                    if i + 1 < NITER:
                        xt_next[q - 1] = emit_A(q - 1)

            for b in range(nR + LAG):
                if b < nR:
                    if b % 4 == 0:
                        t2 = b // 4 + 2
                        if t2 < nT and xt_tiles[t2] is None:
                            xt_tiles[t2] = emit_A(t2, head=(t2 * 512 < HEAD))
                    emit_B(b)
                if b >= LAG:
                    emit_G(b - LAG)
            emit_conv3(nT - 1, wa_sb, b3_sb, i == NITER - 1,
                       head=(i == 0 and (nT - 1) * 512 < HEAD))
            if i + 1 < NITER:
                xt_next[nT - 1] = emit_A(nT - 1)

    nc.compile()
    return nc


def _to_bf16(a):
    return np.asarray(a, dtype=np.float32).astype(ml_dtypes.bfloat16)


def prep_in_maps(x, d, WC, bC, WP, bP, WF, bF, WA, bA, T=T_FULL):
    """Build the 8 per-core input maps from the full-problem arrays.
    Returns (in_maps, has_b1)."""
    x = np.asarray(x, dtype=np.float32)
    d = np.asarray(d, dtype=np.float32)
    WC, WP, WF, WA = (np.asarray(w, dtype=np.float32) for w in (WC, WP, WF, WA))
    bC, bP, bF, bA = (np.asarray(b, dtype=np.float32) for b in (bC, bP, bF, bA))
    nb = x.shape[0]
    nR = T // 128

    wpf = np.empty((NITER, 128, 2, 512), np.float32)
    wct = np.empty((NITER, 128, 2, 256), np.float32)
    wa = np.empty((NITER, 128, 3, 2, 2, 128), np.float32)
    for i in range(NITER):
        wpfT = np.concatenate([WP[i].T, WF[i].T], axis=1)  # [c', 512]
        wpf[i] = wpfT.reshape(2, 128, 512).transpose(1, 0, 2)
        wct[i] = WC[i].T.reshape(2, 128, 256).transpose(1, 0, 2)
        wak = np.empty((3, 2, 2, 128, 128), np.float32)
        for k in range(3):
            waT = WA[i, :, :, k].T                         # [c', o]
            wak[k] = waT.reshape(2, 128, 2, 128).transpose(0, 2, 1, 3)
        wa[i] = wak.transpose(3, 0, 1, 2, 4)               # [p, k, c, o, f]
    b1 = (bC + bP + bF).astype(np.float32)                  # [NITER, 256]
    has_b1 = bool(np.any(b1 != 0))
    b3 = bA.reshape(NITER, 2, 128).transpose(0, 2, 1).astype(np.float32).copy()

    wpf, wct, wa = _to_bf16(wpf), _to_bf16(wct), _to_bf16(wa)
    iota = np.arange(128, dtype=np.float32).reshape(128, 1)
    ident = _to_bf16(np.eye(128, dtype=np.float32))

    tf = np.arange(T, dtype=np.float32)
    in_maps = []
    for b in range(nb):
        dv = d[b, 0].astype(np.float32)
        ixr = np.full((NITER, nR, 512), INVALID, np.int16)
        for i, dil in enumerate(DILATIONS):
            dd = dv * np.float32(dil)
            rp = np.round(tf - dd).astype(np.int64)
            rf = np.round(tf + dd).astype(np.int64)
            rp = np.where(rp >= 0, rp, np.int64(-(1 << 32)))
            rf = np.where(rf < T, rf, np.int64(1 << 32))
            blk = np.arange(nR).repeat(128) * 128           # (T,)
            for c, base in enumerate((rp - blk + 128, rp - blk,
                                      rf - blk, rf - blk - 128)):
                v = np.where((base >= 0) & (base < 128), base,
                             np.int64(INVALID)).astype(np.int16)
                ixr[i, :, c * 128:(c + 1) * 128] = v.reshape(nR, 128)
        xb = x[b].reshape(2, 128, T)
        m = {
            "x": xb.copy(),
            "xhead": xb[:, :, :min(2048, T)].transpose(1, 0, 2).copy(),
            "wpf": wpf, "wct": wct, "wa": wa, "b3": b3,
            "ixrel": np.clip(ixr, -128, 127).astype(np.int8),
            "iota": iota, "ident": ident,
        }
        if has_b1:
            m["b1row"] = _to_bf16(b1.reshape(NITER, 1, 256))
            m["ones"] = _to_bf16(np.ones((1, 128), np.float32))
        in_maps.append(m)
    return in_maps, has_b1


_nc_cache = {}


def kernel(**inputs) -> np.ndarray:
    T = inputs["x"].shape[2]
    in_maps, has_b1 = prep_in_maps(**inputs, T=T)
    key = (T, has_b1)
    if key not in _nc_cache:
        _nc_cache[key] = build_nc(T, has_b1=has_b1)
    nc = _nc_cache[key]
    res = run_bass_kernel_spmd(nc, in_maps, core_ids=list(range(8)))
    out = np.stack([np.asarray(res.results[i]["out"], dtype=np.float32)
                    .reshape(C, T) for i in range(8)])
    return out

